# revision 19
# baseline (speedup 1.0000x reference)
"""Trainium2 Bass kernel for ActorCriticRNN (8-core data-parallel over actors).

Per-core shard: 32 actors, T=64 steps, 2048 images of 6x9x26.
Pipeline (channel-major activations [C, positions]):
  obs --PE-transpose--> [26, pos] -> 1x1 convs c1,c2,c3 (matmul over C) ->
  3x3 convs c4,c5,c6 as: dr-taps stacked into 32-aligned partition strips at
  relu-eviction time, dc-taps as 3 PSUM-accumulated matmuls with shifted rhs
  over a zero-padded free layout (7 rows x 10 cols per image, shared pads) ->
  dense (54 accumulated K=32 matmuls) + LayerNorm (over partitions, via
  ones-matmul stats + K=1 broadcast matmuls) -> GRU (1 step emitted per conv
  timestep, 8 steps behind) -> actor/critic heads per 8-step group.
float32 storage; float32r matmuls (full PE rate at N>=256).
"""

import sys

for _p in ("/opt/trn_rl_repo",):
    if _p not in sys.path:
        sys.path.insert(0, _p)

import numpy as np
import ml_dtypes
BF = ml_dtypes.bfloat16

import concourse.bass as bass
import concourse.bacc as bacc
import concourse.mybir as mybir
import concourse.tile as tile

F32 = mybir.dt.float32
F32R = mybir.dt.float32r
BF16 = mybir.dt.bfloat16
AF = mybir.ActivationFunctionType
ALU = mybir.AluOpType

T, NL, HH, WW, CIN = 64, 32, 6, 9, 26
IMG = T * NL            # 2048
S = HH * WW             # 54
FTOT = IMG * S          # 110592
PW, PH = 10, 7
PS = PW * PH            # 70
PAD0 = 12
TPS = NL * PS           # 2240
PBUF = PAD0 + TPS + 12  # 2264
HID = 128
GRP = 8
GIMG = GRP * NL         # 256
OBST_F = 4 * NL * S     # 6912 (4-timestep ring)
DB_F = 2 * GIMG * S     # 27648 (2-group ring)
CONV_BLKS = [(12 + 490 * k, min(490, 2240 - 490 * k), 7 * k,
              min(7, NL - 7 * k)) for k in range(5)]


def _r(ap):
    return ap.bitcast(F32R)


def build_bass():
    nc = bacc.Bacc(None, target_bir_lowering=False)

    def din(name, shape, dt=F32):
        return nc.declare_dram_parameter(name, list(shape), dt, isOutput=False)

    d = {}
    d["obs"] = din("obs", [FTOT, CIN])
    d["mask1"] = din("mask1", [1, IMG])
    d["h0"] = din("h0", [HID, NL])
    d["c1w"] = din("c1w", [CIN, 128], BF16)
    d["c2w"] = din("c2w", [128, 128])
    d["c3w"] = din("c3w", [128, 8])
    d["w4"] = din("w4", [96, 3, 16], BF16)
    d["w5"] = din("w5", [96, 3, 32], BF16)
    d["w6"] = din("w6", [96, 3, 32], BF16)
    d["dwT"] = din("dwT", [32, S, HID], BF16)
    d["biases"] = din("biases", [128, 16])
    for k in ("wizp", "wirp", "wihp", "whz", "whr", "whh", "afw", "cfw"):
        d[k] = din(k, [HID, HID])
    d["aow"] = din("aow", [HID, 6])
    d["cow"] = din("cow", [HID, 1])
    d["ident"] = din("ident", [128, 128])
    d["identb"] = din("identb", [128, 128], BF16)
    d["ones1x"] = din("ones1x", [1, 128])
    d["onecol"] = din("onecol", [128, 1])
    d["fh_out"] = nc.declare_dram_parameter("final_hidden", [NL, HID], F32,
                                            isOutput=True)
    d["lg_out"] = nc.declare_dram_parameter("logits", [T, NL, 6], F32,
                                            isOutput=True)
    d["vl_out"] = nc.declare_dram_parameter("value", [T, NL], F32,
                                            isOutput=True)

    with tile.TileContext(nc) as tc, \
            nc.allow_low_precision(reason="bf16 mid-layers within rel-err budget"):
        build_body(nc, tc, d)
    if not nc.is_finalized():
        nc.finalize()
    return nc


def build_body(nc, tc, d):
    ctxs = []

    def pool(name, bufs, space="SBUF"):
        p = tc.tile_pool(name=name, bufs=bufs, space=space)
        ctxs.append(p)
        return p.__enter__()

    wpool = pool("weights", 1)
    persist = pool("persist", 1)
    obsp = pool("obs_in", 3)
    xact = pool("xact", 3)
    embp = pool("emb", 2)
    smallp = pool("small", 4)
    outp = pool("outs", 1)
    ps_big = pool("ps_big", 5, space="PSUM")
    ps_sm = pool("ps_sm", 3, space="PSUM")

    def wload(name, shape, dt=F32, rr=False):
        t = wpool.tile(list(shape), dt, tag=name)
        nc.sync.dma_start(out=_r(t) if rr else t,
                          in_=_r(d[name][:]) if rr else d[name][:])
        return t

    c1w = wload("c1w", [CIN, 128], BF16)
    c2w = wload("c2w", [128, 128], rr=True)
    c3w = wload("c3w", [128, 8], rr=True)
    w4 = wload("w4", [96, 3, 16], BF16)
    w5 = wload("w5", [96, 3, 32], BF16)
    w6 = wload("w6", [96, 3, 32], BF16)
    dwT = wload("dwT", [32, S, HID], BF16)
    bia = wload("biases", [128, 16])
    wizp = wload("wizp", [HID, HID], rr=True)
    wirp = wload("wirp", [HID, HID], rr=True)
    wihp = wload("wihp", [HID, HID], rr=True)
    whz = wload("whz", [HID, HID], rr=True)
    whr = wload("whr", [HID, HID], rr=True)
    whh = wload("whh", [HID, HID], rr=True)
    afw = wload("afw", [HID, HID], rr=True)
    aow = wload("aow", [HID, 6], rr=True)
    cfw = wload("cfw", [HID, HID], rr=True)
    cow = wload("cow", [HID, 1], rr=True)
    ident = wload("ident", [128, 128])
    identb = wload("identb", [128, 128], BF16)
    ones1x = wload("ones1x", [1, 128], rr=True)
    onecol = wload("onecol", [128, 1], rr=True)
    epst = wpool.tile([1, 1], F32, tag="epst")
    nc.vector.memset(epst, 1e-6)

    b_c1, b_c2, b_c3 = bia[:, 0:1], bia[:, 1:2], bia[:8, 2:3]
    b_c4, b_c5, b_c6 = bia[:16, 3:4], bia[:32, 4:5], bia[:32, 5:6]
    b_d = bia[:, 6:7]
    b_z, b_r, b_h = bia[:, 7:8], bia[:, 8:9], bia[:, 9:10]
    b_af, b_cf = bia[:, 10:11], bia[:, 11:12]
    b_ao, b_co = bia[:6, 12:13], bia[:1, 13:14]

    obsT = persist.tile([CIN, OBST_F], BF16)
    dbuf = persist.tile([32, DB_F], BF16)
    x4b = persist.tile([96, 2 * PBUF], BF16)
    x5b = persist.tile([96, 2 * PBUF], BF16)
    x6b = persist.tile([96, 2 * PBUF], BF16)
    wizb = persist.tile([HID, IMG], BF16)
    wirb = persist.tile([HID, IMG], BF16)
    wihb = persist.tile([HID, IMG], BF16)
    grub = persist.tile([HID, IMG], F32)
    maskb = persist.tile([HID, IMG], F32)
    h0sb = persist.tile([HID, NL], F32)
    msk1 = persist.tile([1, IMG], F32)

    nc.sync.dma_start(out=h0sb, in_=d["h0"][:])
    nc.sync.dma_start(out=_r(msk1), in_=_r(d["mask1"][:]))

    # one-time pad zeroing of both halves of the padded ping-pong buffers:
    # valid-cell evictions never touch pad cells, so zeros persist.
    for buf in (x4b, x5b, x6b):
        nc.vector.memset(buf, 0.0)

    # mask broadcast to all 128 partitions via K=1 matmuls
    for q in range(IMG // 512):
        psm = ps_big.tile([128, 512], F32, tag="ps")
        nc.tensor.matmul(psm, _r(ones1x), _r(msk1[:, q * 512:(q + 1) * 512]),
                         start=True, stop=True)
        nc.scalar.activation(out=maskb[:, q * 512:(q + 1) * 512], in_=psm,
                             func=AF.Copy)

    obs_r = d["obs"].rearrange("(a j p) c -> a p j c", j=16, p=128)
    state = {"chunk": 0}

    def emit_obs_chunk(a):
        ot = obsp.tile([128, 16, CIN], F32, tag="ot")
        nc.sync.dma_start(out=ot, in_=obs_r[a])
        otb = obsp.tile([128, 16, CIN], BF16, tag="otb")
        nc.vector.tensor_copy(otb, ot)
        for q in range(4):
            pst = ps_sm.tile([26, 512], BF16, tag="pss")
            for j in range(4):
                nc.tensor.transpose(pst[:, j * 128:(j + 1) * 128],
                                    otb[:, q * 4 + j, :], identb)
            base = (a * 2048 + q * 512) % OBST_F
            if base + 512 <= OBST_F:
                nc.vector.tensor_copy(obsT[:, base: base + 512], pst)
            else:
                r = OBST_F - base
                nc.vector.tensor_copy(obsT[:, base: base + r], pst[:, 0:r])
                nc.vector.tensor_copy(obsT[:, 0: 512 - r], pst[:, r:512])

    def valid_ap(buf, plo, phi, f0, nimg):
        v = buf[plo:phi, f0:f0 + nimg * PS]
        return v.rearrange("c (i r w) -> c i r w", r=PH, w=PW)[:, :, 0:6, 0:9]

    def psum_valid(ps, cs, nimg):
        return ps[0:cs, 0:nimg * PS] \
            .rearrange("c (i r w) -> c i r w", r=PH, w=PW)[:, :, 0:6, 0:9]

    def gru_step(tt):
        c0 = tt * NL
        hprev = h0sb if tt == 0 else grub[:, (tt - 1) * NL: tt * NL]
        hm = smallp.tile([HID, NL], F32, tag="hm")
        nc.vector.tensor_mul(_r(hm), hprev, maskb[:, c0:c0 + NL])
        psz = ps_sm.tile([HID, NL], F32, tag="pss")
        nc.tensor.matmul(psz, _r(whz), _r(hm), start=True, stop=False)
        nc.tensor.matmul(psz, identb, wizb[:, c0:c0 + NL],
                         start=False, stop=True)
        psr = ps_sm.tile([HID, NL], F32, tag="pss")
        nc.tensor.matmul(psr, _r(whr), _r(hm), start=True, stop=False)
        nc.tensor.matmul(psr, identb, wirb[:, c0:c0 + NL],
                         start=False, stop=True)
        zz = smallp.tile([HID, NL], F32, tag="zz")
        nc.scalar.activation(out=zz, in_=psz, func=AF.Sigmoid, bias=b_z)
        rr = smallp.tile([HID, NL], F32, tag="rr")
        nc.scalar.activation(out=rr, in_=psr, func=AF.Sigmoid, bias=b_r)
        rh = smallp.tile([HID, NL], F32, tag="rh")
        nc.vector.tensor_mul(_r(rh), rr, hm)
        psh = ps_sm.tile([HID, NL], F32, tag="pss")
        nc.tensor.matmul(psh, _r(whh), _r(rh), start=True, stop=False)
        nc.tensor.matmul(psh, identb, wihb[:, c0:c0 + NL],
                         start=False, stop=True)
        hh = smallp.tile([HID, NL], F32, tag="hh")
        nc.scalar.activation(out=hh, in_=psh, func=AF.Tanh, bias=b_h)
        d1 = smallp.tile([HID, NL], F32, tag="d1")
        nc.vector.tensor_sub(d1, hh, hm)
        nc.vector.tensor_mul(d1, zz, d1)
        nc.vector.tensor_add(_r(grub[:, c0:c0 + NL]), hm, d1)

    def dense_group(g):
        gb = (g % 2) * GIMG * S
        emb = embp.tile([HID, GIMG], F32, tag="emb")
        psd = ps_big.tile([HID, GIMG], F32, tag="ps")
        dbv = dbuf[:, gb:gb + GIMG * S].rearrange("c (i x) -> c i x", x=S)
        for rc in range(S):
            nc.tensor.matmul(psd, dwT[:, rc, :], dbv[:, :, rc],
                             start=(rc == 0), stop=(rc == S - 1))
        nc.scalar.activation(out=_r(emb), in_=psd, func=AF.Relu, bias=b_d)

        sq = embp.tile([HID, GIMG], F32, tag="sq")
        nc.vector.tensor_mul(_r(sq), emb, emb)
        ps_s1 = ps_sm.tile([1, GIMG], F32, tag="pss")
        ps_s2 = ps_sm.tile([1, GIMG], F32, tag="pss")
        nc.tensor.matmul(ps_s1, _r(onecol), _r(emb), start=True, stop=True)
        nc.tensor.matmul(ps_s2, _r(onecol), _r(sq), start=True, stop=True)
        mu = smallp.tile([1, GIMG], F32, tag="mu")
        va = smallp.tile([1, GIMG], F32, tag="va")
        aa = smallp.tile([1, GIMG], F32, tag="aa")
        bb = smallp.tile([1, GIMG], F32, tag="bb")
        nc.vector.tensor_scalar_mul(mu, ps_s1, 1.0 / HID)
        nc.vector.tensor_scalar_mul(va, ps_s2, 1.0 / HID)
        nc.vector.tensor_mul(_r(bb), mu, mu)
        nc.vector.tensor_sub(va, va, bb)
        nc.scalar.activation(out=va, in_=va, func=AF.Sqrt, bias=epst)
        nc.vector.reciprocal(_r(aa), va)
        nc.vector.tensor_mul(_r(bb), mu, aa)
        nc.vector.tensor_scalar_mul(_r(bb), bb, -1.0)
        ps_a = ps_sm.tile([HID, GIMG], F32, tag="pss")
        ps_b = ps_sm.tile([HID, GIMG], F32, tag="pss")
        nc.tensor.matmul(ps_a, _r(ones1x), _r(aa), start=True, stop=True)
        nc.tensor.matmul(ps_b, _r(ones1x), _r(bb), start=True, stop=True)
        nc.vector.tensor_mul(_r(emb), emb, ps_a)
        nc.vector.tensor_add(_r(emb), emb, ps_b)

        for wmat, dstb in ((wizp, wizb), (wirp, wirb), (wihp, wihb)):
            psp = ps_big.tile([HID, GIMG], F32, tag="ps")
            nc.tensor.matmul(psp, _r(wmat), _r(emb), start=True, stop=True)
            nc.scalar.activation(out=dstb[:, g * GIMG:(g + 1) * GIMG],
                                 in_=psp, func=AF.Copy)

    def heads_group(g):
        gsl = grub[:, g * GIMG:(g + 1) * GIMG]
        psa = ps_big.tile([HID, GIMG], F32, tag="ps")
        nc.tensor.matmul(psa, _r(afw), _r(gsl), start=True, stop=True)
        act1 = embp.tile([HID, GIMG], F32, tag="act1")
        nc.scalar.activation(out=_r(act1), in_=psa, func=AF.Relu, bias=b_af)
        psl = ps_sm.tile([6, GIMG], F32, tag="pss")
        nc.tensor.matmul(psl, _r(aow), _r(act1), start=True, stop=True)
        lgs = outp.tile([6, GIMG], F32, tag="lgs")
        nc.vector.tensor_scalar_add(lgs, psl, b_ao)
        lg_view = d["lg_out"].rearrange("t n a -> a (t n)")
        nc.sync.dma_start(out=lg_view[:, g * GIMG:(g + 1) * GIMG], in_=lgs)

        psc = ps_big.tile([HID, GIMG], F32, tag="ps")
        nc.tensor.matmul(psc, _r(cfw), _r(gsl), start=True, stop=True)
        crt1 = embp.tile([HID, GIMG], F32, tag="act1")
        nc.scalar.activation(out=_r(crt1), in_=psc, func=AF.Relu, bias=b_cf)
        psv = ps_sm.tile([1, GIMG], F32, tag="pss")
        nc.tensor.matmul(psv, _r(cow), _r(crt1), start=True, stop=True)
        vls = outp.tile([1, GIMG], F32, tag="vls")
        nc.vector.tensor_scalar_add(vls, psv, b_co)
        vl_view = d["vl_out"].rearrange("t n -> (t n)").unsqueeze(0)
        nc.sync.dma_start(out=vl_view[:, g * GIMG:(g + 1) * GIMG], in_=vls)

    # ------------------------------------------------------------------
    for t in range(T):
        while state["chunk"] * 2048 < min((t + 2) * NL * S, FTOT):
            emit_obs_chunk(state["chunk"])
            state["chunk"] += 1

        ob = (t * NL * S) % OBST_F
        pp = (t % 2) * PBUF  # ping-pong half of padded buffers

        for b in range(4):
            f0 = b * 432
            n = 432
            x1 = xact.tile([128, n], F32, tag="x1")
            x2 = xact.tile([128, n], F32, tag="x2")
            ps1 = ps_big.tile([128, n], F32, tag="ps")
            nc.tensor.matmul(ps1, c1w, obsT[:, ob + f0: ob + f0 + n],
                             start=True, stop=True)
            nc.scalar.activation(out=_r(x1), in_=ps1, func=AF.Relu,
                                 bias=b_c1)
            ps2 = ps_big.tile([128, n], F32, tag="ps")
            nc.tensor.matmul(ps2, _r(c2w), _r(x1),
                             start=True, stop=True)
            nc.vector.tensor_scalar(out=_r(x2), in0=ps2,
                                    scalar1=b_c2, scalar2=0.0,
                                    op0=ALU.add, op1=ALU.max)
            ps3 = ps_big.tile([8, n], F32, tag="ps")
            nc.tensor.matmul(ps3, _r(c3w), _r(x2),
                             start=True, stop=True)
            src = ps3.rearrange("c (i r w) -> c i r w", r=6, w=9)
            f1 = pp + PAD0 + 8 * b * PS
            dst = valid_ap(x4b, 32, 32 + 8, f1, 8)
            nc.scalar.activation(out=dst, in_=src, func=AF.Relu, bias=b_c3)

        def conv3x3(xb, wmat, cs_out, bias_ap, out_cb, compact_dst=None,
                    evict_dve=False):
            for (p0, n, i0, nimg) in CONV_BLKS:
                pso = ps_big.tile([cs_out, 512], F32, tag="ps")
                for dc in range(3):
                    nc.tensor.matmul(
                        pso[:, 0:n], wmat[dc],
                        xb[:, pp + p0 + dc - 1: pp + p0 + dc - 1 + n],
                        start=(dc == 0), stop=(dc == 2))
                srcv = psum_valid(pso, cs_out, nimg)
                if compact_dst is not None:
                    f1 = ((t % 16) * NL + i0) * S
                    dst = compact_dst[:, f1:f1 + nimg * S] \
                        .rearrange("c (i r w) -> c i r w", r=6, w=9)
                    nc.scalar.activation(out=dst, in_=srcv, func=AF.Relu,
                                         bias=bias_ap)
                else:
                    f1 = pp + PAD0 + i0 * PS
                    dst = valid_ap(out_cb, 32, 32 + cs_out, f1, nimg)
                    if evict_dve:
                        nc.vector.tensor_scalar(
                            out=dst, in0=srcv, scalar1=bias_ap,
                            scalar2=0.0, op0=ALU.add, op1=ALU.max)
                    else:
                        nc.scalar.activation(out=dst, in_=srcv,
                                             func=AF.Relu, bias=bias_ap)

        def replicate(buf, cs):
            s0, s1 = pp + PAD0, pp + PAD0 + TPS
            nc.sync.dma_start(out=buf[0:cs, s0 + 10: s1 + 10],
                              in_=buf[32:32 + cs, s0:s1])
            nc.sync.dma_start(out=buf[64:64 + cs, s0 - 10: s1 - 10],
                              in_=buf[32:32 + cs, s0:s1])

        replicate(x4b, 8)
        conv3x3(x4b, [w4[:, dc, :] for dc in range(3)], 16, b_c4, x5b,
                evict_dve=True)
        if t >= GRP:
            gru_step(t - GRP)
        replicate(x5b, 16)
        conv3x3(x5b, [w5[:, dc, :] for dc in range(3)], 32, b_c5, x6b,
                evict_dve=True)
        replicate(x6b, 32)
        conv3x3(x6b, [w6[:, dc, :] for dc in range(3)], 32, b_c6, None,
                compact_dst=dbuf)

        if t % GRP == GRP - 1:
            g = t // GRP
            dense_group(g)
            if g >= 2:
                heads_group(g - 2)

    for tt in range(T - GRP, T):
        gru_step(tt)
    heads_group(6)
    heads_group(7)
    fh_view = d["fh_out"].rearrange("n h -> h n")
    nc.sync.dma_start(out=fh_view, in_=grub[:, (T - 1) * NL: T * NL])

    for p in reversed(ctxs):
        p.__exit__(None, None, None)


# ---------------------------------------------------------------------------
_NC_CACHE = {}


def _get_nc():
    if "nc" not in _NC_CACHE:
        _NC_CACHE["nc"] = build_bass()
    return _NC_CACHE["nc"]


def _prep_weights(kw):
    f = np.float32
    kw = {k: np.asarray(v) for k, v in kw.items()}
    out = {}
    out["c1w"] = kw["c1w"].reshape(26, 128).astype(BF)
    out["c2w"] = kw["c2w"].reshape(128, 128).astype(f)
    out["c3w"] = kw["c3w"].reshape(128, 8).astype(f)
    for nm, key, ci, co in (("w4", "c4w", 8, 16), ("w5", "c5w", 16, 32),
                            ("w6", "c6w", 32, 32)):
        w = np.zeros((96, 3, co), f)
        src = kw[key]
        for dc in range(3):
            for dr in range(3):
                w[32 * dr: 32 * dr + ci, dc, :] = src[dr, dc]
        out[nm] = w.astype(BF)
    out["dwT"] = kw["dw"].reshape(6, 9, 32, 128).transpose(2, 0, 1, 3) \
                          .reshape(32, S, 128).astype(BF).copy()
    lns, lnb = kw["lns"].astype(f), kw["lnb"].astype(f)
    out["wizp"] = (kw["wizk"] * lns[:, None]).astype(f)
    out["wirp"] = (kw["wirk"] * lns[:, None]).astype(f)
    out["wihp"] = (kw["wihk"] * lns[:, None]).astype(f)
    for k in ("whz", "whr", "whh", "afw", "aow", "cfw", "cow"):
        out[k] = kw[k].astype(f)
    bz = (kw["bz"] + kw["wizk"].T @ lnb).astype(f)
    br = (kw["br"] + kw["wirk"].T @ lnb).astype(f)
    bh = (kw["bh"] + kw["wihk"].T @ lnb).astype(f)
    bias = np.zeros((128, 16), f)
    for k, j in (("c1b", 0), ("c2b", 1), ("c3b", 2), ("c4b", 3), ("c5b", 4),
                 ("c6b", 5), ("db", 6)):
        v = kw[k]
        bias[: v.shape[0], j] = v
    bias[:, 7], bias[:, 8], bias[:, 9] = bz, br, bh
    bias[:, 10], bias[:, 11] = kw["afb"], kw["cfb"]
    bias[:6, 12] = kw["aob"]
    bias[:1, 13] = kw["cob"]
    out["biases"] = bias
    out["ident"] = np.eye(128, dtype=f)
    out["identb"] = np.eye(128, dtype=BF)
    out["ones1x"] = np.ones((1, 128), f)
    out["onecol"] = np.ones((128, 1), f)
    return out


def make_in_maps(inputs):
    wmap = _prep_weights({k: v for k, v in inputs.items()
                          if k not in ("obs", "hidden", "dones")})
    obs = np.asarray(inputs["obs"], np.float32)
    hidden = np.asarray(inputs["hidden"], np.float32)
    dones = np.asarray(inputs["dones"])
    in_maps = []
    for c in range(8):
        sl = slice(c * NL, (c + 1) * NL)
        m = dict(wmap)
        m["obs"] = np.ascontiguousarray(obs[:, sl]).reshape(FTOT, CIN)
        m["mask1"] = np.ascontiguousarray(
            1.0 - dones[:, sl].astype(np.float32)).reshape(1, IMG)
        m["h0"] = np.ascontiguousarray(hidden[sl].T)
        in_maps.append(m)
    return in_maps


def kernel(**inputs):
    from concourse.bass_utils import run_bass_kernel_spmd

    nc = _get_nc()
    in_maps = make_in_maps(inputs)
    res = run_bass_kernel_spmd(nc, in_maps, core_ids=list(range(8)))
    outs = res.results
    fh = np.concatenate([outs[c]["final_hidden"] for c in range(8)], 0)
    lg = np.concatenate([outs[c]["logits"] for c in range(8)], 1)
    vl = np.concatenate([outs[c]["value"] for c in range(8)], 1)
    return fh, lg, vl


if __name__ == "__main__":
    nc = build_bass()
    print("build ok")


# revision 20
# speedup vs baseline: 1.3561x; 1.3561x over previous
"""Trainium2 Bass kernel for ActorCriticRNN (8-core data-parallel over actors).

Per-core shard: 32 actors, T=64 steps, 2048 images of 6x9x26.
Pipeline (channel-major activations [C, positions]):
  obs --PE-transpose--> [26, pos] -> 1x1 convs c1,c2,c3 (matmul over C) ->
  3x3 convs c4,c5,c6 as: dr-taps stacked into 32-aligned partition strips at
  relu-eviction time, dc-taps as 3 PSUM-accumulated matmuls with shifted rhs
  over a zero-padded free layout (7 rows x 10 cols per image, shared pads) ->
  dense (54 accumulated K=32 matmuls) + LayerNorm (over partitions, via
  ones-matmul stats + K=1 broadcast matmuls) -> GRU (1 step emitted per conv
  timestep, 8 steps behind) -> actor/critic heads per 8-step group.
float32 storage; float32r matmuls (full PE rate at N>=256).
"""

import sys

for _p in ("/opt/trn_rl_repo",):
    if _p not in sys.path:
        sys.path.insert(0, _p)

import numpy as np
import ml_dtypes
BF = ml_dtypes.bfloat16

import concourse.bass as bass
import concourse.bacc as bacc
import concourse.mybir as mybir
import concourse.tile as tile

F32 = mybir.dt.float32
F32R = mybir.dt.float32r
BF16 = mybir.dt.bfloat16
AF = mybir.ActivationFunctionType
ALU = mybir.AluOpType

T, NL, HH, WW, CIN = 64, 32, 6, 9, 26
IMG = T * NL            # 2048
S = HH * WW             # 54
FTOT = IMG * S          # 110592
PW, PH = 10, 7
PS = PW * PH            # 70
PAD0 = 12
TPS = NL * PS           # 2240
PBUF = PAD0 + TPS + 12  # 2264
HID = 128
GRP = 8
GIMG = GRP * NL         # 256
OBST_F = 4 * NL * S     # 6912 (4-timestep ring)
DB_F = 2 * GIMG * S     # 27648 (2-group ring)
CONV_BLKS = [(12 + 490 * k, min(490, 2240 - 490 * k), 7 * k,
              min(7, NL - 7 * k)) for k in range(5)]


def _r(ap):
    return ap.bitcast(F32R)


def build_bass():
    nc = bacc.Bacc(None, target_bir_lowering=False)

    def din(name, shape, dt=F32):
        return nc.declare_dram_parameter(name, list(shape), dt, isOutput=False)

    d = {}
    d["obs"] = din("obs", [FTOT, CIN])
    d["mask1"] = din("mask1", [1, IMG])
    d["h0"] = din("h0", [HID, NL])
    d["c1w"] = din("c1w", [CIN, 128], BF16)
    d["c2w"] = din("c2w", [128, 128])
    d["c3w"] = din("c3w", [128, 8])
    d["w4"] = din("w4", [96, 3, 16], BF16)
    d["w5"] = din("w5", [96, 3, 32], BF16)
    d["w6"] = din("w6", [96, 3, 32], BF16)
    d["dwT"] = din("dwT", [32, S, HID], BF16)
    d["biases"] = din("biases", [128, 16])
    for k in ("wizp", "wirp", "wihp", "whz", "whr", "whh", "afw", "cfw"):
        d[k] = din(k, [HID, HID])
    d["aow"] = din("aow", [HID, 6])
    d["cow"] = din("cow", [HID, 1])
    d["ident"] = din("ident", [128, 128])
    d["identb"] = din("identb", [128, 128], BF16)
    d["ones1x"] = din("ones1x", [1, 128])
    d["onecol"] = din("onecol", [128, 1])
    d["fh_out"] = nc.declare_dram_parameter("final_hidden", [NL, HID], F32,
                                            isOutput=True)
    d["lg_out"] = nc.declare_dram_parameter("logits", [T, NL, 6], F32,
                                            isOutput=True)
    d["vl_out"] = nc.declare_dram_parameter("value", [T, NL], F32,
                                            isOutput=True)

    with tile.TileContext(nc) as tc, \
            nc.allow_low_precision(reason="bf16 mid-layers within rel-err budget"):
        build_body(nc, tc, d)
    if not nc.is_finalized():
        nc.finalize()
    return nc


def build_body(nc, tc, d):
    ctxs = []

    def pool(name, bufs, space="SBUF"):
        p = tc.tile_pool(name=name, bufs=bufs, space=space)
        ctxs.append(p)
        return p.__enter__()

    wpool = pool("weights", 1)
    persist = pool("persist", 1)
    obsp = pool("obs_in", 3)
    xact = pool("xact", 3)
    embp = pool("emb", 2)
    smallp = pool("small", 4)
    outp = pool("outs", 1)
    ps_big = pool("ps_big", 5, space="PSUM")
    ps_sm = pool("ps_sm", 3, space="PSUM")

    def wload(name, shape, dt=F32, rr=False):
        t = wpool.tile(list(shape), dt, tag=name)
        nc.sync.dma_start(out=_r(t) if rr else t,
                          in_=_r(d[name][:]) if rr else d[name][:])
        return t

    c1w = wload("c1w", [CIN, 128], BF16)
    c2w = wload("c2w", [128, 128], rr=True)
    c3w = wload("c3w", [128, 8], rr=True)
    w4 = wload("w4", [96, 3, 16], BF16)
    w5 = wload("w5", [96, 3, 32], BF16)
    w6 = wload("w6", [96, 3, 32], BF16)
    dwT = wload("dwT", [32, S, HID], BF16)
    bia = wload("biases", [128, 16])
    wizp = wload("wizp", [HID, HID], rr=True)
    wirp = wload("wirp", [HID, HID], rr=True)
    wihp = wload("wihp", [HID, HID], rr=True)
    whz = wload("whz", [HID, HID], rr=True)
    whr = wload("whr", [HID, HID], rr=True)
    whh = wload("whh", [HID, HID], rr=True)
    afw = wload("afw", [HID, HID], rr=True)
    aow = wload("aow", [HID, 6], rr=True)
    cfw = wload("cfw", [HID, HID], rr=True)
    cow = wload("cow", [HID, 1], rr=True)
    ident = wload("ident", [128, 128])
    identb = wload("identb", [128, 128], BF16)
    ones1x = wload("ones1x", [1, 128], rr=True)
    onecol = wload("onecol", [128, 1], rr=True)
    epst = wpool.tile([1, 1], F32, tag="epst")
    nc.vector.memset(epst, 1e-6)

    b_c1, b_c2, b_c3 = bia[:, 0:1], bia[:, 1:2], bia[:8, 2:3]
    b_c4, b_c5, b_c6 = bia[:16, 3:4], bia[:32, 4:5], bia[:32, 5:6]
    b_d = bia[:, 6:7]
    b_z, b_r, b_h = bia[:, 7:8], bia[:, 8:9], bia[:, 9:10]
    b_af, b_cf = bia[:, 10:11], bia[:, 11:12]
    b_ao, b_co = bia[:6, 12:13], bia[:1, 13:14]

    obsT = persist.tile([CIN, OBST_F], BF16)
    dbuf = persist.tile([32, DB_F], BF16)
    x4b = persist.tile([96, 2 * PBUF], BF16)
    x5b = persist.tile([96, 2 * PBUF], BF16)
    x6b = persist.tile([96, 2 * PBUF], BF16)
    wizb = persist.tile([HID, IMG], BF16)
    wirb = persist.tile([HID, IMG], BF16)
    wihb = persist.tile([HID, IMG], BF16)
    grub = persist.tile([HID, IMG], F32)
    maskb = persist.tile([HID, IMG], F32)
    h0sb = persist.tile([HID, NL], F32)
    msk1 = persist.tile([1, IMG], F32)

    nc.sync.dma_start(out=h0sb, in_=d["h0"][:])
    nc.sync.dma_start(out=_r(msk1), in_=_r(d["mask1"][:]))

    # one-time pad zeroing of both halves of the padded ping-pong buffers:
    # valid-cell evictions never touch pad cells, so zeros persist.
    for buf in (x4b, x5b, x6b):
        nc.vector.memset(buf, 0.0)

    # mask broadcast to all 128 partitions via K=1 matmuls
    for q in range(IMG // 512):
        psm = ps_big.tile([128, 512], F32, tag="ps")
        nc.tensor.matmul(psm, _r(ones1x), _r(msk1[:, q * 512:(q + 1) * 512]),
                         start=True, stop=True)
        nc.scalar.activation(out=maskb[:, q * 512:(q + 1) * 512], in_=psm,
                             func=AF.Copy)

    obs_r = d["obs"].rearrange("(a j p) c -> a p j c", j=16, p=128)
    state = {"chunk": 0}

    def emit_obs_chunk(a):
        ot = obsp.tile([128, 16, CIN], F32, tag="ot")
        nc.sync.dma_start(out=ot, in_=obs_r[a])
        otb = obsp.tile([128, 16, CIN], BF16, tag="otb")
        nc.vector.tensor_copy(otb, ot)
        for q in range(4):
            pst = ps_sm.tile([26, 512], BF16, tag="pss")
            for j in range(4):
                nc.tensor.transpose(pst[:, j * 128:(j + 1) * 128],
                                    otb[:, q * 4 + j, :], identb)
            base = (a * 2048 + q * 512) % OBST_F
            if base + 512 <= OBST_F:
                nc.vector.tensor_copy(obsT[:, base: base + 512], pst)
            else:
                r = OBST_F - base
                nc.vector.tensor_copy(obsT[:, base: base + r], pst[:, 0:r])
                nc.vector.tensor_copy(obsT[:, 0: 512 - r], pst[:, r:512])

    def valid_ap(buf, plo, phi, f0, nimg):
        v = buf[plo:phi, f0:f0 + nimg * PS]
        return v.rearrange("c (i r w) -> c i r w", r=PH, w=PW)[:, :, 0:6, 0:9]

    def psum_valid(ps, cs, nimg):
        return ps[0:cs, 0:nimg * PS] \
            .rearrange("c (i r w) -> c i r w", r=PH, w=PW)[:, :, 0:6, 0:9]

    def gru_step(tt):
        c0 = tt * NL
        hprev = h0sb if tt == 0 else grub[:, (tt - 1) * NL: tt * NL]
        hm = smallp.tile([HID, NL], F32, tag="hm")
        nc.vector.tensor_mul(_r(hm), hprev, maskb[:, c0:c0 + NL])
        psz = ps_sm.tile([HID, NL], F32, tag="pss")
        nc.tensor.matmul(psz, _r(whz), _r(hm), start=True, stop=False)
        nc.tensor.matmul(psz, identb, wizb[:, c0:c0 + NL],
                         start=False, stop=True)
        psr = ps_sm.tile([HID, NL], F32, tag="pss")
        nc.tensor.matmul(psr, _r(whr), _r(hm), start=True, stop=False)
        nc.tensor.matmul(psr, identb, wirb[:, c0:c0 + NL],
                         start=False, stop=True)
        zz = smallp.tile([HID, NL], F32, tag="zz")
        nc.scalar.activation(out=zz, in_=psz, func=AF.Sigmoid, bias=b_z)
        rr = smallp.tile([HID, NL], F32, tag="rr")
        nc.scalar.activation(out=rr, in_=psr, func=AF.Sigmoid, bias=b_r)
        rh = smallp.tile([HID, NL], F32, tag="rh")
        nc.vector.tensor_mul(_r(rh), rr, hm)
        psh = ps_sm.tile([HID, NL], F32, tag="pss")
        nc.tensor.matmul(psh, _r(whh), _r(rh), start=True, stop=False)
        nc.tensor.matmul(psh, identb, wihb[:, c0:c0 + NL],
                         start=False, stop=True)
        hh = smallp.tile([HID, NL], F32, tag="hh")
        nc.scalar.activation(out=hh, in_=psh, func=AF.Tanh, bias=b_h)
        d1 = smallp.tile([HID, NL], F32, tag="d1")
        nc.vector.tensor_sub(d1, hh, hm)
        nc.vector.tensor_mul(d1, zz, d1)
        nc.vector.tensor_add(_r(grub[:, c0:c0 + NL]), hm, d1)

    def dense_group(g):
        gb = (g % 2) * GIMG * S
        emb = embp.tile([HID, GIMG], F32, tag="emb")
        psd = ps_big.tile([HID, GIMG], F32, tag="ps")
        dbv = dbuf[:, gb:gb + GIMG * S].rearrange("c (i x) -> c i x", x=S)
        for rc in range(S):
            nc.tensor.matmul(psd, dwT[:, rc, :], dbv[:, :, rc],
                             start=(rc == 0), stop=(rc == S - 1))
        nc.scalar.activation(out=_r(emb), in_=psd, func=AF.Relu, bias=b_d)

        sq = embp.tile([HID, GIMG], F32, tag="sq")
        nc.vector.tensor_mul(_r(sq), emb, emb)
        ps_s1 = ps_sm.tile([1, GIMG], F32, tag="pss")
        ps_s2 = ps_sm.tile([1, GIMG], F32, tag="pss")
        nc.tensor.matmul(ps_s1, _r(onecol), _r(emb), start=True, stop=True)
        nc.tensor.matmul(ps_s2, _r(onecol), _r(sq), start=True, stop=True)
        mu = smallp.tile([1, GIMG], F32, tag="mu")
        va = smallp.tile([1, GIMG], F32, tag="va")
        aa = smallp.tile([1, GIMG], F32, tag="aa")
        bb = smallp.tile([1, GIMG], F32, tag="bb")
        nc.vector.tensor_scalar_mul(mu, ps_s1, 1.0 / HID)
        nc.vector.tensor_scalar_mul(va, ps_s2, 1.0 / HID)
        nc.vector.tensor_mul(_r(bb), mu, mu)
        nc.vector.tensor_sub(va, va, bb)
        nc.scalar.activation(out=va, in_=va, func=AF.Sqrt, bias=epst)
        nc.vector.reciprocal(_r(aa), va)
        nc.vector.tensor_mul(_r(bb), mu, aa)
        nc.vector.tensor_scalar_mul(_r(bb), bb, -1.0)
        ps_a = ps_sm.tile([HID, GIMG], F32, tag="pss")
        ps_b = ps_sm.tile([HID, GIMG], F32, tag="pss")
        nc.tensor.matmul(ps_a, _r(ones1x), _r(aa), start=True, stop=True)
        nc.tensor.matmul(ps_b, _r(ones1x), _r(bb), start=True, stop=True)
        nc.vector.tensor_mul(_r(emb), emb, ps_a)
        nc.vector.tensor_add(_r(emb), emb, ps_b)

        for wmat, dstb in ((wizp, wizb), (wirp, wirb), (wihp, wihb)):
            psp = ps_big.tile([HID, GIMG], F32, tag="ps")
            nc.tensor.matmul(psp, _r(wmat), _r(emb), start=True, stop=True)
            nc.scalar.activation(out=dstb[:, g * GIMG:(g + 1) * GIMG],
                                 in_=psp, func=AF.Copy)

    def heads_group(g):
        gsl = grub[:, g * GIMG:(g + 1) * GIMG]
        psa = ps_big.tile([HID, GIMG], F32, tag="ps")
        nc.tensor.matmul(psa, _r(afw), _r(gsl), start=True, stop=True)
        act1 = embp.tile([HID, GIMG], F32, tag="act1")
        nc.scalar.activation(out=_r(act1), in_=psa, func=AF.Relu, bias=b_af)
        psl = ps_sm.tile([6, GIMG], F32, tag="pss")
        nc.tensor.matmul(psl, _r(aow), _r(act1), start=True, stop=True)
        lgs = outp.tile([6, GIMG], F32, tag="lgs")
        nc.vector.tensor_scalar_add(lgs, psl, b_ao)
        lg_view = d["lg_out"].rearrange("t n a -> a (t n)")
        nc.sync.dma_start(out=lg_view[:, g * GIMG:(g + 1) * GIMG], in_=lgs)

        psc = ps_big.tile([HID, GIMG], F32, tag="ps")
        nc.tensor.matmul(psc, _r(cfw), _r(gsl), start=True, stop=True)
        crt1 = embp.tile([HID, GIMG], F32, tag="act1")
        nc.scalar.activation(out=_r(crt1), in_=psc, func=AF.Relu, bias=b_cf)
        psv = ps_sm.tile([1, GIMG], F32, tag="pss")
        nc.tensor.matmul(psv, _r(cow), _r(crt1), start=True, stop=True)
        vls = outp.tile([1, GIMG], F32, tag="vls")
        nc.vector.tensor_scalar_add(vls, psv, b_co)
        vl_view = d["vl_out"].rearrange("t n -> (t n)").unsqueeze(0)
        nc.sync.dma_start(out=vl_view[:, g * GIMG:(g + 1) * GIMG], in_=vls)

    # ------------------------------------------------------------------
    for t in range(T):
        while state["chunk"] * 2048 < min((t + 2) * NL * S, FTOT):
            emit_obs_chunk(state["chunk"])
            state["chunk"] += 1

        ob = (t * NL * S) % OBST_F
        pp = (t % 2) * PBUF  # ping-pong half of padded buffers

        for b in range(4):
            f0 = b * 432
            n = 432
            x1 = xact.tile([128, n], F32, tag="x1")
            x2 = xact.tile([128, n], F32, tag="x2")
            ps1 = ps_big.tile([128, n], F32, tag="ps")
            nc.tensor.matmul(ps1, c1w, obsT[:, ob + f0: ob + f0 + n],
                             start=True, stop=True)
            nc.scalar.activation(out=_r(x1), in_=ps1, func=AF.Relu,
                                 bias=b_c1)
            ps2 = ps_big.tile([128, n], F32, tag="ps")
            nc.tensor.matmul(ps2, _r(c2w), _r(x1),
                             start=True, stop=True)
            nc.vector.tensor_scalar(out=_r(x2), in0=ps2,
                                    scalar1=b_c2, scalar2=0.0,
                                    op0=ALU.add, op1=ALU.max)
            ps3 = ps_big.tile([8, n], F32, tag="ps")
            nc.tensor.matmul(ps3, _r(c3w), _r(x2),
                             start=True, stop=True)
            src = ps3.rearrange("c (i r w) -> c i r w", r=6, w=9)
            for dr in range(3):
                f1 = pp + PAD0 + 8 * b * PS - 10 * (dr - 1)
                dst = valid_ap(x4b, 32 * dr, 32 * dr + 8, f1, 8)
                nc.scalar.activation(out=dst, in_=src, func=AF.Relu, bias=b_c3)

        def conv3x3(xb, wmat, cs_out, bias_ap, out_cb, compact_dst=None,
                    evict_dve=False):
            for (p0, n, i0, nimg) in CONV_BLKS:
                pso = ps_big.tile([cs_out, 512], F32, tag="ps")
                for dc in range(3):
                    nc.tensor.matmul(
                        pso[:, 0:n], wmat[dc],
                        xb[:, pp + p0 + dc - 1: pp + p0 + dc - 1 + n],
                        start=(dc == 0), stop=(dc == 2))
                srcv = psum_valid(pso, cs_out, nimg)
                if compact_dst is not None:
                    f1 = ((t % 16) * NL + i0) * S
                    dst = compact_dst[:, f1:f1 + nimg * S] \
                        .rearrange("c (i r w) -> c i r w", r=6, w=9)
                    nc.scalar.activation(out=dst, in_=srcv, func=AF.Relu,
                                         bias=bias_ap)
                else:
                    for dr in range(3):
                        f1 = pp + PAD0 + i0 * PS - 10 * (dr - 1)
                        dst = valid_ap(out_cb, 32 * dr, 32 * dr + cs_out,
                                       f1, nimg)
                        if evict_dve and dr == 0:
                            nc.vector.tensor_scalar(
                                out=dst, in0=srcv, scalar1=bias_ap,
                                scalar2=0.0, op0=ALU.add, op1=ALU.max)
                        else:
                            nc.scalar.activation(out=dst, in_=srcv,
                                                 func=AF.Relu, bias=bias_ap)

        conv3x3(x4b, [w4[:, dc, :] for dc in range(3)], 16, b_c4, x5b,
                evict_dve=True)
        if t >= GRP:
            gru_step(t - GRP)
        conv3x3(x5b, [w5[:, dc, :] for dc in range(3)], 32, b_c5, x6b,
                evict_dve=True)
        conv3x3(x6b, [w6[:, dc, :] for dc in range(3)], 32, b_c6, None,
                compact_dst=dbuf)

        if t % GRP == GRP - 1:
            g = t // GRP
            dense_group(g)
            if g >= 2:
                heads_group(g - 2)

    for tt in range(T - GRP, T):
        gru_step(tt)
    heads_group(6)
    heads_group(7)
    fh_view = d["fh_out"].rearrange("n h -> h n")
    nc.sync.dma_start(out=fh_view, in_=grub[:, (T - 1) * NL: T * NL])

    for p in reversed(ctxs):
        p.__exit__(None, None, None)


# ---------------------------------------------------------------------------
_NC_CACHE = {}


def _get_nc():
    if "nc" not in _NC_CACHE:
        _NC_CACHE["nc"] = build_bass()
    return _NC_CACHE["nc"]


def _prep_weights(kw):
    f = np.float32
    kw = {k: np.asarray(v) for k, v in kw.items()}
    out = {}
    out["c1w"] = kw["c1w"].reshape(26, 128).astype(BF)
    out["c2w"] = kw["c2w"].reshape(128, 128).astype(f)
    out["c3w"] = kw["c3w"].reshape(128, 8).astype(f)
    for nm, key, ci, co in (("w4", "c4w", 8, 16), ("w5", "c5w", 16, 32),
                            ("w6", "c6w", 32, 32)):
        w = np.zeros((96, 3, co), f)
        src = kw[key]
        for dc in range(3):
            for dr in range(3):
                w[32 * dr: 32 * dr + ci, dc, :] = src[dr, dc]
        out[nm] = w.astype(BF)
    out["dwT"] = kw["dw"].reshape(6, 9, 32, 128).transpose(2, 0, 1, 3) \
                          .reshape(32, S, 128).astype(BF).copy()
    lns, lnb = kw["lns"].astype(f), kw["lnb"].astype(f)
    out["wizp"] = (kw["wizk"] * lns[:, None]).astype(f)
    out["wirp"] = (kw["wirk"] * lns[:, None]).astype(f)
    out["wihp"] = (kw["wihk"] * lns[:, None]).astype(f)
    for k in ("whz", "whr", "whh", "afw", "aow", "cfw", "cow"):
        out[k] = kw[k].astype(f)
    bz = (kw["bz"] + kw["wizk"].T @ lnb).astype(f)
    br = (kw["br"] + kw["wirk"].T @ lnb).astype(f)
    bh = (kw["bh"] + kw["wihk"].T @ lnb).astype(f)
    bias = np.zeros((128, 16), f)
    for k, j in (("c1b", 0), ("c2b", 1), ("c3b", 2), ("c4b", 3), ("c5b", 4),
                 ("c6b", 5), ("db", 6)):
        v = kw[k]
        bias[: v.shape[0], j] = v
    bias[:, 7], bias[:, 8], bias[:, 9] = bz, br, bh
    bias[:, 10], bias[:, 11] = kw["afb"], kw["cfb"]
    bias[:6, 12] = kw["aob"]
    bias[:1, 13] = kw["cob"]
    out["biases"] = bias
    out["ident"] = np.eye(128, dtype=f)
    out["identb"] = np.eye(128, dtype=BF)
    out["ones1x"] = np.ones((1, 128), f)
    out["onecol"] = np.ones((128, 1), f)
    return out


def make_in_maps(inputs):
    wmap = _prep_weights({k: v for k, v in inputs.items()
                          if k not in ("obs", "hidden", "dones")})
    obs = np.asarray(inputs["obs"], np.float32)
    hidden = np.asarray(inputs["hidden"], np.float32)
    dones = np.asarray(inputs["dones"])
    in_maps = []
    for c in range(8):
        sl = slice(c * NL, (c + 1) * NL)
        m = dict(wmap)
        m["obs"] = np.ascontiguousarray(obs[:, sl]).reshape(FTOT, CIN)
        m["mask1"] = np.ascontiguousarray(
            1.0 - dones[:, sl].astype(np.float32)).reshape(1, IMG)
        m["h0"] = np.ascontiguousarray(hidden[sl].T)
        in_maps.append(m)
    return in_maps


def kernel(**inputs):
    from concourse.bass_utils import run_bass_kernel_spmd

    nc = _get_nc()
    in_maps = make_in_maps(inputs)
    res = run_bass_kernel_spmd(nc, in_maps, core_ids=list(range(8)))
    outs = res.results
    fh = np.concatenate([outs[c]["final_hidden"] for c in range(8)], 0)
    lg = np.concatenate([outs[c]["logits"] for c in range(8)], 1)
    vl = np.concatenate([outs[c]["value"] for c in range(8)], 1)
    return fh, lg, vl


if __name__ == "__main__":
    nc = build_bass()
    print("build ok")


# revision 21
# speedup vs baseline: 1.3997x; 1.0321x over previous
"""Trainium2 Bass kernel for ActorCriticRNN (8-core data-parallel over actors).

Per-core shard: 32 actors, T=64 steps, 2048 images of 6x9x26.
Pipeline (channel-major activations [C, positions]):
  obs --PE-transpose--> [26, pos] -> 1x1 convs c1,c2,c3 (matmul over C) ->
  3x3 convs c4,c5,c6 as: dr-taps stacked into 32-aligned partition strips at
  relu-eviction time, dc-taps as 3 PSUM-accumulated matmuls with shifted rhs
  over a zero-padded free layout (7 rows x 10 cols per image, shared pads) ->
  dense (54 accumulated K=32 matmuls) + LayerNorm (over partitions, via
  ones-matmul stats + K=1 broadcast matmuls) -> GRU (1 step emitted per conv
  timestep, 8 steps behind) -> actor/critic heads per 8-step group.
float32 storage; float32r matmuls (full PE rate at N>=256).
"""

import sys

for _p in ("/opt/trn_rl_repo",):
    if _p not in sys.path:
        sys.path.insert(0, _p)

import numpy as np
import ml_dtypes
BF = ml_dtypes.bfloat16

import concourse.bass as bass
import concourse.bacc as bacc
import concourse.mybir as mybir
import concourse.tile as tile

F32 = mybir.dt.float32
F32R = mybir.dt.float32r
BF16 = mybir.dt.bfloat16
AF = mybir.ActivationFunctionType
ALU = mybir.AluOpType

T, NL, HH, WW, CIN = 64, 32, 6, 9, 26
IMG = T * NL            # 2048
S = HH * WW             # 54
FTOT = IMG * S          # 110592
PW, PH = 10, 7
PS = PW * PH            # 70
PAD0 = 12
TPS = NL * PS           # 2240
PBUF = PAD0 + TPS + 12  # 2264
HID = 128
GRP = 8
GIMG = GRP * NL         # 256
OBST_F = 4 * NL * S     # 6912 (4-timestep ring)
DB_F = 2 * GIMG * S     # 27648 (2-group ring)
CONV_BLKS = [(12 + 490 * k, min(490, 2240 - 490 * k), 7 * k,
              min(7, NL - 7 * k)) for k in range(5)]


def _r(ap):
    return ap.bitcast(F32R)


def build_bass():
    nc = bacc.Bacc(None, target_bir_lowering=False)

    def din(name, shape, dt=F32):
        return nc.declare_dram_parameter(name, list(shape), dt, isOutput=False)

    d = {}
    d["obs"] = din("obs", [FTOT, CIN])
    d["mask1"] = din("mask1", [1, IMG])
    d["h0"] = din("h0", [HID, NL])
    d["c1w"] = din("c1w", [CIN, 128], BF16)
    d["c2w"] = din("c2w", [128, 128], BF16)
    d["c3w"] = din("c3w", [128, 8], BF16)
    d["w4"] = din("w4", [96, 3, 16], BF16)
    d["w5"] = din("w5", [96, 3, 32], BF16)
    d["w6"] = din("w6", [96, 3, 32], BF16)
    d["dwT"] = din("dwT", [32, S, HID], BF16)
    d["biases"] = din("biases", [128, 16])
    for k in ("wizp", "wirp", "wihp", "whz", "whr", "whh", "afw", "cfw"):
        d[k] = din(k, [HID, HID])
    d["aow"] = din("aow", [HID, 6])
    d["cow"] = din("cow", [HID, 1])
    d["ident"] = din("ident", [128, 128])
    d["identb"] = din("identb", [128, 128], BF16)
    d["ones1x"] = din("ones1x", [1, 128])
    d["onecol"] = din("onecol", [128, 1])
    d["fh_out"] = nc.declare_dram_parameter("final_hidden", [NL, HID], F32,
                                            isOutput=True)
    d["lg_out"] = nc.declare_dram_parameter("logits", [T, NL, 6], F32,
                                            isOutput=True)
    d["vl_out"] = nc.declare_dram_parameter("value", [T, NL], F32,
                                            isOutput=True)

    with tile.TileContext(nc) as tc, \
            nc.allow_low_precision(reason="bf16 mid-layers within rel-err budget"):
        build_body(nc, tc, d)
    if not nc.is_finalized():
        nc.finalize()
    return nc


def build_body(nc, tc, d):
    ctxs = []

    def pool(name, bufs, space="SBUF"):
        p = tc.tile_pool(name=name, bufs=bufs, space=space)
        ctxs.append(p)
        return p.__enter__()

    wpool = pool("weights", 1)
    persist = pool("persist", 1)
    obsp = pool("obs_in", 3)
    xact = pool("xact", 3)
    embp = pool("emb", 2)
    smallp = pool("small", 4)
    outp = pool("outs", 1)
    ps_big = pool("ps_big", 5, space="PSUM")
    ps_sm = pool("ps_sm", 3, space="PSUM")

    def wload(name, shape, dt=F32, rr=False):
        t = wpool.tile(list(shape), dt, tag=name)
        nc.sync.dma_start(out=_r(t) if rr else t,
                          in_=_r(d[name][:]) if rr else d[name][:])
        return t

    c1w = wload("c1w", [CIN, 128], BF16)
    c2w = wload("c2w", [128, 128], BF16)
    c3w = wload("c3w", [128, 8], BF16)
    w4 = wload("w4", [96, 3, 16], BF16)
    w5 = wload("w5", [96, 3, 32], BF16)
    w6 = wload("w6", [96, 3, 32], BF16)
    dwT = wload("dwT", [32, S, HID], BF16)
    bia = wload("biases", [128, 16])
    wizp = wload("wizp", [HID, HID], rr=True)
    wirp = wload("wirp", [HID, HID], rr=True)
    wihp = wload("wihp", [HID, HID], rr=True)
    whz = wload("whz", [HID, HID], rr=True)
    whr = wload("whr", [HID, HID], rr=True)
    whh = wload("whh", [HID, HID], rr=True)
    afw = wload("afw", [HID, HID], rr=True)
    aow = wload("aow", [HID, 6], rr=True)
    cfw = wload("cfw", [HID, HID], rr=True)
    cow = wload("cow", [HID, 1], rr=True)
    ident = wload("ident", [128, 128])
    identb = wload("identb", [128, 128], BF16)
    ones1x = wload("ones1x", [1, 128], rr=True)
    onecol = wload("onecol", [128, 1], rr=True)
    epst = wpool.tile([1, 1], F32, tag="epst")
    nc.vector.memset(epst, 1e-6)

    b_c1, b_c2, b_c3 = bia[:, 0:1], bia[:, 1:2], bia[:8, 2:3]
    b_c4, b_c5, b_c6 = bia[:16, 3:4], bia[:32, 4:5], bia[:32, 5:6]
    b_d = bia[:, 6:7]
    b_z, b_r, b_h = bia[:, 7:8], bia[:, 8:9], bia[:, 9:10]
    b_af, b_cf = bia[:, 10:11], bia[:, 11:12]
    b_ao, b_co = bia[:6, 12:13], bia[:1, 13:14]

    obsT = persist.tile([CIN, OBST_F], BF16)
    dbuf = persist.tile([32, DB_F], BF16)
    x4b = persist.tile([96, 2 * PBUF], BF16)
    x5b = persist.tile([96, 2 * PBUF], BF16)
    x6b = persist.tile([96, 2 * PBUF], BF16)
    wizb = persist.tile([HID, IMG], BF16)
    wirb = persist.tile([HID, IMG], BF16)
    wihb = persist.tile([HID, IMG], BF16)
    grub = persist.tile([HID, IMG], F32)
    maskb = persist.tile([HID, IMG], F32)
    h0sb = persist.tile([HID, NL], F32)
    msk1 = persist.tile([1, IMG], F32)

    nc.sync.dma_start(out=h0sb, in_=d["h0"][:])
    nc.sync.dma_start(out=_r(msk1), in_=_r(d["mask1"][:]))

    # one-time pad zeroing of both halves of the padded ping-pong buffers:
    # valid-cell evictions never touch pad cells, so zeros persist.
    for buf in (x4b, x5b, x6b):
        nc.vector.memset(buf, 0.0)

    # mask broadcast to all 128 partitions via K=1 matmuls
    for q in range(IMG // 512):
        psm = ps_big.tile([128, 512], F32, tag="ps")
        nc.tensor.matmul(psm, _r(ones1x), _r(msk1[:, q * 512:(q + 1) * 512]),
                         start=True, stop=True)
        nc.scalar.activation(out=maskb[:, q * 512:(q + 1) * 512], in_=psm,
                             func=AF.Copy)

    obs_r = d["obs"].rearrange("(a j p) c -> a p j c", j=16, p=128)
    state = {"chunk": 0}

    def emit_obs_chunk(a):
        ot = obsp.tile([128, 16, CIN], F32, tag="ot")
        nc.sync.dma_start(out=ot, in_=obs_r[a])
        otb = obsp.tile([128, 16, CIN], BF16, tag="otb")
        nc.vector.tensor_copy(otb, ot)
        for q in range(4):
            pst = ps_sm.tile([26, 512], BF16, tag="pss")
            for j in range(4):
                nc.tensor.transpose(pst[:, j * 128:(j + 1) * 128],
                                    otb[:, q * 4 + j, :], identb)
            base = (a * 2048 + q * 512) % OBST_F
            if base + 512 <= OBST_F:
                nc.vector.tensor_copy(obsT[:, base: base + 512], pst)
            else:
                r = OBST_F - base
                nc.vector.tensor_copy(obsT[:, base: base + r], pst[:, 0:r])
                nc.vector.tensor_copy(obsT[:, 0: 512 - r], pst[:, r:512])

    def valid_ap(buf, plo, phi, f0, nimg):
        v = buf[plo:phi, f0:f0 + nimg * PS]
        return v.rearrange("c (i r w) -> c i r w", r=PH, w=PW)[:, :, 0:6, 0:9]

    def psum_valid(ps, cs, nimg):
        return ps[0:cs, 0:nimg * PS] \
            .rearrange("c (i r w) -> c i r w", r=PH, w=PW)[:, :, 0:6, 0:9]

    def gru_step(tt):
        c0 = tt * NL
        hprev = h0sb if tt == 0 else grub[:, (tt - 1) * NL: tt * NL]
        hm = smallp.tile([HID, NL], F32, tag="hm")
        nc.vector.tensor_mul(_r(hm), hprev, maskb[:, c0:c0 + NL])
        psz = ps_sm.tile([HID, NL], F32, tag="pss")
        nc.tensor.matmul(psz, _r(whz), _r(hm), start=True, stop=False)
        nc.tensor.matmul(psz, identb, wizb[:, c0:c0 + NL],
                         start=False, stop=True)
        psr = ps_sm.tile([HID, NL], F32, tag="pss")
        nc.tensor.matmul(psr, _r(whr), _r(hm), start=True, stop=False)
        nc.tensor.matmul(psr, identb, wirb[:, c0:c0 + NL],
                         start=False, stop=True)
        zz = smallp.tile([HID, NL], F32, tag="zz")
        nc.scalar.activation(out=zz, in_=psz, func=AF.Sigmoid, bias=b_z)
        rr = smallp.tile([HID, NL], F32, tag="rr")
        nc.scalar.activation(out=rr, in_=psr, func=AF.Sigmoid, bias=b_r)
        rh = smallp.tile([HID, NL], F32, tag="rh")
        nc.vector.tensor_mul(_r(rh), rr, hm)
        psh = ps_sm.tile([HID, NL], F32, tag="pss")
        nc.tensor.matmul(psh, _r(whh), _r(rh), start=True, stop=False)
        nc.tensor.matmul(psh, identb, wihb[:, c0:c0 + NL],
                         start=False, stop=True)
        hh = smallp.tile([HID, NL], F32, tag="hh")
        nc.scalar.activation(out=hh, in_=psh, func=AF.Tanh, bias=b_h)
        d1 = smallp.tile([HID, NL], F32, tag="d1")
        nc.vector.tensor_sub(d1, hh, hm)
        nc.vector.tensor_mul(d1, zz, d1)
        nc.vector.tensor_add(_r(grub[:, c0:c0 + NL]), hm, d1)

    def dense_group(g):
        gb = (g % 2) * GIMG * S
        emb = embp.tile([HID, GIMG], F32, tag="emb")
        psd = ps_big.tile([HID, GIMG], F32, tag="ps")
        for rc in range(S):
            nc.tensor.matmul(psd, dwT[:, rc, :],
                             dbuf[:, gb + rc * GIMG: gb + (rc + 1) * GIMG],
                             start=(rc == 0), stop=(rc == S - 1))
        nc.scalar.activation(out=_r(emb), in_=psd, func=AF.Relu, bias=b_d)

        sq = embp.tile([HID, GIMG], F32, tag="sq")
        nc.vector.tensor_mul(_r(sq), emb, emb)
        ps_s1 = ps_sm.tile([1, GIMG], F32, tag="pss")
        ps_s2 = ps_sm.tile([1, GIMG], F32, tag="pss")
        nc.tensor.matmul(ps_s1, _r(onecol), _r(emb), start=True, stop=True)
        nc.tensor.matmul(ps_s2, _r(onecol), _r(sq), start=True, stop=True)
        mu = smallp.tile([1, GIMG], F32, tag="mu")
        va = smallp.tile([1, GIMG], F32, tag="va")
        aa = smallp.tile([1, GIMG], F32, tag="aa")
        bb = smallp.tile([1, GIMG], F32, tag="bb")
        nc.vector.tensor_scalar_mul(mu, ps_s1, 1.0 / HID)
        nc.vector.tensor_scalar_mul(va, ps_s2, 1.0 / HID)
        nc.vector.tensor_mul(_r(bb), mu, mu)
        nc.vector.tensor_sub(va, va, bb)
        nc.scalar.activation(out=va, in_=va, func=AF.Sqrt, bias=epst)
        nc.vector.reciprocal(_r(aa), va)
        nc.vector.tensor_mul(_r(bb), mu, aa)
        nc.vector.tensor_scalar_mul(_r(bb), bb, -1.0)
        ps_a = ps_sm.tile([HID, GIMG], F32, tag="pss")
        ps_b = ps_sm.tile([HID, GIMG], F32, tag="pss")
        nc.tensor.matmul(ps_a, _r(ones1x), _r(aa), start=True, stop=True)
        nc.tensor.matmul(ps_b, _r(ones1x), _r(bb), start=True, stop=True)
        nc.vector.tensor_mul(_r(emb), emb, ps_a)
        nc.vector.tensor_add(_r(emb), emb, ps_b)

        for wmat, dstb in ((wizp, wizb), (wirp, wirb), (wihp, wihb)):
            psp = ps_big.tile([HID, GIMG], F32, tag="ps")
            nc.tensor.matmul(psp, _r(wmat), _r(emb), start=True, stop=True)
            nc.scalar.activation(out=dstb[:, g * GIMG:(g + 1) * GIMG],
                                 in_=psp, func=AF.Copy)

    def heads_group(g):
        gsl = grub[:, g * GIMG:(g + 1) * GIMG]
        psa = ps_big.tile([HID, GIMG], F32, tag="ps")
        nc.tensor.matmul(psa, _r(afw), _r(gsl), start=True, stop=True)
        act1 = embp.tile([HID, GIMG], F32, tag="act1")
        nc.scalar.activation(out=_r(act1), in_=psa, func=AF.Relu, bias=b_af)
        psl = ps_sm.tile([6, GIMG], F32, tag="pss")
        nc.tensor.matmul(psl, _r(aow), _r(act1), start=True, stop=True)
        lgs = outp.tile([6, GIMG], F32, tag="lgs")
        nc.vector.tensor_scalar_add(lgs, psl, b_ao)
        lg_view = d["lg_out"].rearrange("t n a -> a (t n)")
        nc.sync.dma_start(out=lg_view[:, g * GIMG:(g + 1) * GIMG], in_=lgs)

        psc = ps_big.tile([HID, GIMG], F32, tag="ps")
        nc.tensor.matmul(psc, _r(cfw), _r(gsl), start=True, stop=True)
        crt1 = embp.tile([HID, GIMG], F32, tag="act1")
        nc.scalar.activation(out=_r(crt1), in_=psc, func=AF.Relu, bias=b_cf)
        psv = ps_sm.tile([1, GIMG], F32, tag="pss")
        nc.tensor.matmul(psv, _r(cow), _r(crt1), start=True, stop=True)
        vls = outp.tile([1, GIMG], F32, tag="vls")
        nc.vector.tensor_scalar_add(vls, psv, b_co)
        vl_view = d["vl_out"].rearrange("t n -> (t n)").unsqueeze(0)
        nc.sync.dma_start(out=vl_view[:, g * GIMG:(g + 1) * GIMG], in_=vls)

    # ------------------------------------------------------------------
    for t in range(T):
        while state["chunk"] * 2048 < min((t + 2) * NL * S, FTOT):
            emit_obs_chunk(state["chunk"])
            state["chunk"] += 1

        ob = (t * NL * S) % OBST_F
        pp = (t % 2) * PBUF  # ping-pong half of padded buffers

        for b in range(4):
            f0 = b * 432
            n = 432
            x1 = xact.tile([128, n], BF16, tag="x1")
            x2 = xact.tile([128, n], BF16, tag="x2")
            ps1 = ps_big.tile([128, n], F32, tag="ps")
            nc.tensor.matmul(ps1, c1w, obsT[:, ob + f0: ob + f0 + n],
                             start=True, stop=True)
            nc.scalar.activation(out=x1, in_=ps1, func=AF.Relu,
                                 bias=b_c1)
            ps2 = ps_big.tile([128, n], F32, tag="ps")
            nc.tensor.matmul(ps2, c2w, x1,
                             start=True, stop=True)
            nc.vector.tensor_scalar(out=x2, in0=ps2,
                                    scalar1=b_c2, scalar2=0.0,
                                    op0=ALU.add, op1=ALU.max)
            ps3 = ps_big.tile([8, n], F32, tag="ps")
            nc.tensor.matmul(ps3, c3w, x2,
                             start=True, stop=True)
            src = ps3.rearrange("c (i r w) -> c i r w", r=6, w=9)
            for dr in range(3):
                f1 = pp + PAD0 + 8 * b * PS - 10 * (dr - 1)
                dst = valid_ap(x4b, 32 * dr, 32 * dr + 8, f1, 8)
                if dr == 0:
                    nc.vector.tensor_scalar(out=dst, in0=src, scalar1=b_c3,
                                            scalar2=0.0, op0=ALU.add,
                                            op1=ALU.max)
                else:
                    nc.scalar.activation(out=dst, in_=src, func=AF.Relu,
                                         bias=b_c3)

        def conv3x3(xb, wmat, cs_out, bias_ap, out_cb, compact_dst=None,
                    evict_dve=False):
            for (p0, n, i0, nimg) in CONV_BLKS:
                pso = ps_big.tile([cs_out, 512], F32, tag="ps")
                for dc in range(3):
                    nc.tensor.matmul(
                        pso[:, 0:n], wmat[dc],
                        xb[:, pp + p0 + dc - 1: pp + p0 + dc - 1 + n],
                        start=(dc == 0), stop=(dc == 2))
                srcv = psum_valid(pso, cs_out, nimg)
                if compact_dst is not None:
                    gb2 = ((t // 8) % 2) * GIMG * S
                    ig = (t % 8) * NL + i0
                    v = compact_dst[:, gb2:gb2 + GIMG * S] \
                        .rearrange("c (rw i) -> c rw i", i=GIMG) \
                        .rearrange("c (r w) i -> c i r w", w=9)
                    dst = v[:, ig:ig + nimg, :, :]
                    nc.vector.tensor_scalar(out=dst, in0=srcv, scalar1=bias_ap,
                                            scalar2=0.0, op0=ALU.add,
                                            op1=ALU.max)
                else:
                    for dr in range(3):
                        f1 = pp + PAD0 + i0 * PS - 10 * (dr - 1)
                        dst = valid_ap(out_cb, 32 * dr, 32 * dr + cs_out,
                                       f1, nimg)
                        if evict_dve and dr == 0:
                            nc.vector.tensor_scalar(
                                out=dst, in0=srcv, scalar1=bias_ap,
                                scalar2=0.0, op0=ALU.add, op1=ALU.max)
                        else:
                            nc.scalar.activation(out=dst, in_=srcv,
                                                 func=AF.Relu, bias=bias_ap)

        conv3x3(x4b, [w4[:, dc, :] for dc in range(3)], 16, b_c4, x5b,
                evict_dve=True)
        if t >= GRP:
            gru_step(t - GRP)
        conv3x3(x5b, [w5[:, dc, :] for dc in range(3)], 32, b_c5, x6b,
                evict_dve=True)
        conv3x3(x6b, [w6[:, dc, :] for dc in range(3)], 32, b_c6, None,
                compact_dst=dbuf)

        if t % GRP == GRP - 1:
            g = t // GRP
            dense_group(g)
            if g >= 2:
                heads_group(g - 2)

    for tt in range(T - GRP, T):
        gru_step(tt)
    heads_group(6)
    heads_group(7)
    fh_view = d["fh_out"].rearrange("n h -> h n")
    nc.sync.dma_start(out=fh_view, in_=grub[:, (T - 1) * NL: T * NL])

    for p in reversed(ctxs):
        p.__exit__(None, None, None)


# ---------------------------------------------------------------------------
_NC_CACHE = {}


def _get_nc():
    if "nc" not in _NC_CACHE:
        _NC_CACHE["nc"] = build_bass()
    return _NC_CACHE["nc"]


def _prep_weights(kw):
    f = np.float32
    kw = {k: np.asarray(v) for k, v in kw.items()}
    out = {}
    out["c1w"] = kw["c1w"].reshape(26, 128).astype(BF)
    out["c2w"] = kw["c2w"].reshape(128, 128).astype(BF)
    out["c3w"] = kw["c3w"].reshape(128, 8).astype(BF)
    for nm, key, ci, co in (("w4", "c4w", 8, 16), ("w5", "c5w", 16, 32),
                            ("w6", "c6w", 32, 32)):
        w = np.zeros((96, 3, co), f)
        src = kw[key]
        for dc in range(3):
            for dr in range(3):
                w[32 * dr: 32 * dr + ci, dc, :] = src[dr, dc]
        out[nm] = w.astype(BF)
    out["dwT"] = kw["dw"].reshape(6, 9, 32, 128).transpose(2, 0, 1, 3) \
                          .reshape(32, S, 128).astype(BF).copy()
    lns, lnb = kw["lns"].astype(f), kw["lnb"].astype(f)
    out["wizp"] = (kw["wizk"] * lns[:, None]).astype(f)
    out["wirp"] = (kw["wirk"] * lns[:, None]).astype(f)
    out["wihp"] = (kw["wihk"] * lns[:, None]).astype(f)
    for k in ("whz", "whr", "whh", "afw", "aow", "cfw", "cow"):
        out[k] = kw[k].astype(f)
    bz = (kw["bz"] + kw["wizk"].T @ lnb).astype(f)
    br = (kw["br"] + kw["wirk"].T @ lnb).astype(f)
    bh = (kw["bh"] + kw["wihk"].T @ lnb).astype(f)
    bias = np.zeros((128, 16), f)
    for k, j in (("c1b", 0), ("c2b", 1), ("c3b", 2), ("c4b", 3), ("c5b", 4),
                 ("c6b", 5), ("db", 6)):
        v = kw[k]
        bias[: v.shape[0], j] = v
    bias[:, 7], bias[:, 8], bias[:, 9] = bz, br, bh
    bias[:, 10], bias[:, 11] = kw["afb"], kw["cfb"]
    bias[:6, 12] = kw["aob"]
    bias[:1, 13] = kw["cob"]
    out["biases"] = bias
    out["ident"] = np.eye(128, dtype=f)
    out["identb"] = np.eye(128, dtype=BF)
    out["ones1x"] = np.ones((1, 128), f)
    out["onecol"] = np.ones((128, 1), f)
    return out


def make_in_maps(inputs):
    wmap = _prep_weights({k: v for k, v in inputs.items()
                          if k not in ("obs", "hidden", "dones")})
    obs = np.asarray(inputs["obs"], np.float32)
    hidden = np.asarray(inputs["hidden"], np.float32)
    dones = np.asarray(inputs["dones"])
    in_maps = []
    for c in range(8):
        sl = slice(c * NL, (c + 1) * NL)
        m = dict(wmap)
        m["obs"] = np.ascontiguousarray(obs[:, sl]).reshape(FTOT, CIN)
        m["mask1"] = np.ascontiguousarray(
            1.0 - dones[:, sl].astype(np.float32)).reshape(1, IMG)
        m["h0"] = np.ascontiguousarray(hidden[sl].T)
        in_maps.append(m)
    return in_maps


def kernel(**inputs):
    from concourse.bass_utils import run_bass_kernel_spmd

    nc = _get_nc()
    in_maps = make_in_maps(inputs)
    res = run_bass_kernel_spmd(nc, in_maps, core_ids=list(range(8)))
    outs = res.results
    fh = np.concatenate([outs[c]["final_hidden"] for c in range(8)], 0)
    lg = np.concatenate([outs[c]["logits"] for c in range(8)], 1)
    vl = np.concatenate([outs[c]["value"] for c in range(8)], 1)
    return fh, lg, vl


if __name__ == "__main__":
    nc = build_bass()
    print("build ok")


# revision 22
# speedup vs baseline: 1.4033x; 1.0026x over previous
"""Trainium2 Bass kernel for ActorCriticRNN (8-core data-parallel over actors).

Per-core shard: 32 actors, T=64 steps, 2048 images of 6x9x26.
Pipeline (channel-major activations [C, positions]):
  obs --PE-transpose--> [26, pos] -> 1x1 convs c1,c2,c3 (matmul over C) ->
  3x3 convs c4,c5,c6 as: dr-taps stacked into 32-aligned partition strips at
  relu-eviction time, dc-taps as 3 PSUM-accumulated matmuls with shifted rhs
  over a zero-padded free layout (7 rows x 10 cols per image, shared pads) ->
  dense (54 accumulated K=32 matmuls) + LayerNorm (over partitions, via
  ones-matmul stats + K=1 broadcast matmuls) -> GRU (1 step emitted per conv
  timestep, 8 steps behind) -> actor/critic heads per 8-step group.
float32 storage; float32r matmuls (full PE rate at N>=256).
"""

import sys

for _p in ("/opt/trn_rl_repo",):
    if _p not in sys.path:
        sys.path.insert(0, _p)

import numpy as np
import ml_dtypes
BF = ml_dtypes.bfloat16

import concourse.bass as bass
import concourse.bacc as bacc
import concourse.mybir as mybir
import concourse.tile as tile

F32 = mybir.dt.float32
F32R = mybir.dt.float32r
BF16 = mybir.dt.bfloat16
AF = mybir.ActivationFunctionType
ALU = mybir.AluOpType

T, NL, HH, WW, CIN = 64, 32, 6, 9, 26
IMG = T * NL            # 2048
S = HH * WW             # 54
FTOT = IMG * S          # 110592
PW, PH = 10, 7
PS = PW * PH            # 70
PAD0 = 12
TPS = NL * PS           # 2240
PBUF = PAD0 + TPS + 12  # 2264
HID = 128
GRP = 8
GIMG = GRP * NL         # 256
OBST_F = 4 * NL * S     # 6912 (4-timestep ring)
DB_F = 2 * GIMG * S     # 27648 (2-group ring)
CONV_BLKS = [(12 + 490 * k, min(490, 2240 - 490 * k), 7 * k,
              min(7, NL - 7 * k)) for k in range(5)]


def _r(ap):
    return ap.bitcast(F32R)


def build_bass():
    nc = bacc.Bacc(None, target_bir_lowering=False)

    def din(name, shape, dt=F32):
        return nc.declare_dram_parameter(name, list(shape), dt, isOutput=False)

    d = {}
    d["obs"] = din("obs", [FTOT, CIN])
    d["mask1"] = din("mask1", [1, IMG])
    d["h0"] = din("h0", [HID, NL])
    d["c1w"] = din("c1w", [CIN, 128], BF16)
    d["c2w"] = din("c2w", [128, 128], BF16)
    d["c3w"] = din("c3w", [128, 8], BF16)
    d["w4"] = din("w4", [96, 3, 16], BF16)
    d["w5"] = din("w5", [96, 3, 32], BF16)
    d["w6"] = din("w6", [96, 3, 32], BF16)
    d["dwT"] = din("dwT", [32, S, HID], BF16)
    d["biases"] = din("biases", [128, 16])
    for k in ("wizp", "wirp", "wihp", "whz", "whr", "whh", "afw", "cfw"):
        d[k] = din(k, [HID, HID])
    d["aow"] = din("aow", [HID, 6])
    d["cow"] = din("cow", [HID, 1])
    d["ident"] = din("ident", [128, 128])
    d["identb"] = din("identb", [128, 128], BF16)
    d["ones1x"] = din("ones1x", [1, 128])
    d["onecol"] = din("onecol", [128, 1])
    d["fh_out"] = nc.declare_dram_parameter("final_hidden", [NL, HID], F32,
                                            isOutput=True)
    d["lg_out"] = nc.declare_dram_parameter("logits", [T, NL, 6], F32,
                                            isOutput=True)
    d["vl_out"] = nc.declare_dram_parameter("value", [T, NL], F32,
                                            isOutput=True)

    with tile.TileContext(nc) as tc, \
            nc.allow_low_precision(reason="bf16 mid-layers within rel-err budget"):
        build_body(nc, tc, d)
    if not nc.is_finalized():
        nc.finalize()
    return nc


def build_body(nc, tc, d):
    ctxs = []

    def pool(name, bufs, space="SBUF"):
        p = tc.tile_pool(name=name, bufs=bufs, space=space)
        ctxs.append(p)
        return p.__enter__()

    wpool = pool("weights", 1)
    persist = pool("persist", 1)
    obsp = pool("obs_in", 4)
    xact = pool("xact", 3)
    embp = pool("emb", 2)
    smallp = pool("small", 4)
    outp = pool("outs", 1)
    ps_big = pool("ps_big", 6, space="PSUM")
    ps_sm = pool("ps_sm", 2, space="PSUM")

    def wload(name, shape, dt=F32, rr=False):
        t = wpool.tile(list(shape), dt, tag=name)
        nc.sync.dma_start(out=_r(t) if rr else t,
                          in_=_r(d[name][:]) if rr else d[name][:])
        return t

    c1w = wload("c1w", [CIN, 128], BF16)
    c2w = wload("c2w", [128, 128], BF16)
    c3w = wload("c3w", [128, 8], BF16)
    w4 = wload("w4", [96, 3, 16], BF16)
    w5 = wload("w5", [96, 3, 32], BF16)
    w6 = wload("w6", [96, 3, 32], BF16)
    dwT = wload("dwT", [32, S, HID], BF16)
    bia = wload("biases", [128, 16])
    wizp = wload("wizp", [HID, HID], rr=True)
    wirp = wload("wirp", [HID, HID], rr=True)
    wihp = wload("wihp", [HID, HID], rr=True)
    whz = wload("whz", [HID, HID], rr=True)
    whr = wload("whr", [HID, HID], rr=True)
    whh = wload("whh", [HID, HID], rr=True)
    afw = wload("afw", [HID, HID], rr=True)
    aow = wload("aow", [HID, 6], rr=True)
    cfw = wload("cfw", [HID, HID], rr=True)
    cow = wload("cow", [HID, 1], rr=True)
    ident = wload("ident", [128, 128])
    identb = wload("identb", [128, 128], BF16)
    ones1x = wload("ones1x", [1, 128], rr=True)
    onecol = wload("onecol", [128, 1], rr=True)
    epst = wpool.tile([1, 1], F32, tag="epst")
    nc.vector.memset(epst, 1e-6)

    b_c1, b_c2, b_c3 = bia[:, 0:1], bia[:, 1:2], bia[:8, 2:3]
    b_c4, b_c5, b_c6 = bia[:16, 3:4], bia[:32, 4:5], bia[:32, 5:6]
    b_d = bia[:, 6:7]
    b_z, b_r, b_h = bia[:, 7:8], bia[:, 8:9], bia[:, 9:10]
    b_af, b_cf = bia[:, 10:11], bia[:, 11:12]
    b_ao, b_co = bia[:6, 12:13], bia[:1, 13:14]

    obsT = persist.tile([CIN, OBST_F], BF16)
    dbuf = persist.tile([32, DB_F], BF16)
    x4b = persist.tile([96, 2 * PBUF], BF16)
    x5b = persist.tile([96, 2 * PBUF], BF16)
    x6b = persist.tile([96, 2 * PBUF], BF16)
    wizb = persist.tile([HID, IMG], BF16)
    wirb = persist.tile([HID, IMG], BF16)
    wihb = persist.tile([HID, IMG], BF16)
    grub = persist.tile([HID, IMG], F32)
    maskb = persist.tile([HID, IMG], F32)
    h0sb = persist.tile([HID, NL], F32)
    msk1 = persist.tile([1, IMG], F32)

    nc.sync.dma_start(out=h0sb, in_=d["h0"][:])
    nc.sync.dma_start(out=_r(msk1), in_=_r(d["mask1"][:]))

    # one-time pad zeroing of both halves of the padded ping-pong buffers:
    # valid-cell evictions never touch pad cells, so zeros persist.
    for buf in (x4b, x5b, x6b):
        nc.vector.memset(buf, 0.0)

    # mask broadcast to all 128 partitions via K=1 matmuls
    for q in range(IMG // 512):
        psm = ps_big.tile([128, 512], F32, tag="ps")
        nc.tensor.matmul(psm, _r(ones1x), _r(msk1[:, q * 512:(q + 1) * 512]),
                         start=True, stop=True)
        nc.scalar.activation(out=maskb[:, q * 512:(q + 1) * 512], in_=psm,
                             func=AF.Copy)

    obs_r = d["obs"].rearrange("(a j p) c -> a p j c", j=16, p=128)
    state = {"chunk": 0}

    def emit_obs_chunk(a):
        ot = obsp.tile([128, 16, CIN], F32, tag="ot")
        nc.sync.dma_start(out=ot, in_=obs_r[a])
        otb = obsp.tile([128, 16, CIN], BF16, tag="otb")
        nc.vector.tensor_copy(otb, ot)
        for q in range(4):
            pst = ps_sm.tile([26, 512], BF16, tag="pss")
            for j in range(4):
                nc.tensor.transpose(pst[:, j * 128:(j + 1) * 128],
                                    otb[:, q * 4 + j, :], identb)
            base = (a * 2048 + q * 512) % OBST_F
            if base + 512 <= OBST_F:
                nc.vector.tensor_copy(obsT[:, base: base + 512], pst)
            else:
                r = OBST_F - base
                nc.vector.tensor_copy(obsT[:, base: base + r], pst[:, 0:r])
                nc.vector.tensor_copy(obsT[:, 0: 512 - r], pst[:, r:512])

    def valid_ap(buf, plo, phi, f0, nimg):
        v = buf[plo:phi, f0:f0 + nimg * PS]
        return v.rearrange("c (i r w) -> c i r w", r=PH, w=PW)[:, :, 0:6, 0:9]

    def psum_valid(ps, cs, nimg):
        return ps[0:cs, 0:nimg * PS] \
            .rearrange("c (i r w) -> c i r w", r=PH, w=PW)[:, :, 0:6, 0:9]

    def gru_step(tt):
        c0 = tt * NL
        hprev = h0sb if tt == 0 else grub[:, (tt - 1) * NL: tt * NL]
        hm = smallp.tile([HID, NL], F32, tag="hm")
        nc.vector.tensor_mul(_r(hm), hprev, maskb[:, c0:c0 + NL])
        psz = ps_sm.tile([HID, NL], F32, tag="pss")
        nc.tensor.matmul(psz, _r(whz), _r(hm), start=True, stop=False)
        nc.tensor.matmul(psz, identb, wizb[:, c0:c0 + NL],
                         start=False, stop=True)
        psr = ps_sm.tile([HID, NL], F32, tag="pss")
        nc.tensor.matmul(psr, _r(whr), _r(hm), start=True, stop=False)
        nc.tensor.matmul(psr, identb, wirb[:, c0:c0 + NL],
                         start=False, stop=True)
        zz = smallp.tile([HID, NL], F32, tag="zz")
        nc.scalar.activation(out=zz, in_=psz, func=AF.Sigmoid, bias=b_z)
        rr = smallp.tile([HID, NL], F32, tag="rr")
        nc.scalar.activation(out=rr, in_=psr, func=AF.Sigmoid, bias=b_r)
        rh = smallp.tile([HID, NL], F32, tag="rh")
        nc.vector.tensor_mul(_r(rh), rr, hm)
        psh = ps_sm.tile([HID, NL], F32, tag="pss")
        nc.tensor.matmul(psh, _r(whh), _r(rh), start=True, stop=False)
        nc.tensor.matmul(psh, identb, wihb[:, c0:c0 + NL],
                         start=False, stop=True)
        hh = smallp.tile([HID, NL], F32, tag="hh")
        nc.scalar.activation(out=hh, in_=psh, func=AF.Tanh, bias=b_h)
        d1 = smallp.tile([HID, NL], F32, tag="d1")
        nc.vector.tensor_sub(d1, hh, hm)
        nc.vector.tensor_mul(d1, zz, d1)
        nc.vector.tensor_add(_r(grub[:, c0:c0 + NL]), hm, d1)

    def dense_group(g):
        gb = (g % 2) * GIMG * S
        emb = embp.tile([HID, GIMG], F32, tag="emb")
        psd = ps_big.tile([HID, GIMG], F32, tag="ps")
        for rc in range(S):
            nc.tensor.matmul(psd, dwT[:, rc, :],
                             dbuf[:, gb + rc * GIMG: gb + (rc + 1) * GIMG],
                             start=(rc == 0), stop=(rc == S - 1))
        nc.scalar.activation(out=_r(emb), in_=psd, func=AF.Relu, bias=b_d)

        sq = embp.tile([HID, GIMG], F32, tag="sq")
        nc.vector.tensor_mul(_r(sq), emb, emb)
        ps_s1 = ps_sm.tile([1, GIMG], F32, tag="pss")
        ps_s2 = ps_sm.tile([1, GIMG], F32, tag="pss")
        nc.tensor.matmul(ps_s1, _r(onecol), _r(emb), start=True, stop=True)
        nc.tensor.matmul(ps_s2, _r(onecol), _r(sq), start=True, stop=True)
        mu = smallp.tile([1, GIMG], F32, tag="mu")
        va = smallp.tile([1, GIMG], F32, tag="va")
        aa = smallp.tile([1, GIMG], F32, tag="aa")
        bb = smallp.tile([1, GIMG], F32, tag="bb")
        nc.vector.tensor_scalar_mul(mu, ps_s1, 1.0 / HID)
        nc.vector.tensor_scalar_mul(va, ps_s2, 1.0 / HID)
        nc.vector.tensor_mul(_r(bb), mu, mu)
        nc.vector.tensor_sub(va, va, bb)
        nc.scalar.activation(out=va, in_=va, func=AF.Sqrt, bias=epst)
        nc.vector.reciprocal(_r(aa), va)
        nc.vector.tensor_mul(_r(bb), mu, aa)
        nc.vector.tensor_scalar_mul(_r(bb), bb, -1.0)
        ps_a = ps_sm.tile([HID, GIMG], F32, tag="pss")
        ps_b = ps_sm.tile([HID, GIMG], F32, tag="pss")
        nc.tensor.matmul(ps_a, _r(ones1x), _r(aa), start=True, stop=True)
        nc.tensor.matmul(ps_b, _r(ones1x), _r(bb), start=True, stop=True)
        nc.vector.tensor_mul(_r(emb), emb, ps_a)
        nc.vector.tensor_add(_r(emb), emb, ps_b)

        for wmat, dstb in ((wizp, wizb), (wirp, wirb), (wihp, wihb)):
            psp = ps_big.tile([HID, GIMG], F32, tag="ps")
            nc.tensor.matmul(psp, _r(wmat), _r(emb), start=True, stop=True)
            nc.scalar.activation(out=dstb[:, g * GIMG:(g + 1) * GIMG],
                                 in_=psp, func=AF.Copy)

    def heads_group(g):
        gsl = grub[:, g * GIMG:(g + 1) * GIMG]
        psa = ps_big.tile([HID, GIMG], F32, tag="ps")
        nc.tensor.matmul(psa, _r(afw), _r(gsl), start=True, stop=True)
        act1 = embp.tile([HID, GIMG], F32, tag="act1")
        nc.scalar.activation(out=_r(act1), in_=psa, func=AF.Relu, bias=b_af)
        psl = ps_sm.tile([6, GIMG], F32, tag="pss")
        nc.tensor.matmul(psl, _r(aow), _r(act1), start=True, stop=True)
        lgs = outp.tile([6, GIMG], F32, tag="lgs")
        nc.vector.tensor_scalar_add(lgs, psl, b_ao)
        lg_view = d["lg_out"].rearrange("t n a -> a (t n)")
        nc.sync.dma_start(out=lg_view[:, g * GIMG:(g + 1) * GIMG], in_=lgs)

        psc = ps_big.tile([HID, GIMG], F32, tag="ps")
        nc.tensor.matmul(psc, _r(cfw), _r(gsl), start=True, stop=True)
        crt1 = embp.tile([HID, GIMG], F32, tag="act1")
        nc.scalar.activation(out=_r(crt1), in_=psc, func=AF.Relu, bias=b_cf)
        psv = ps_sm.tile([1, GIMG], F32, tag="pss")
        nc.tensor.matmul(psv, _r(cow), _r(crt1), start=True, stop=True)
        vls = outp.tile([1, GIMG], F32, tag="vls")
        nc.vector.tensor_scalar_add(vls, psv, b_co)
        vl_view = d["vl_out"].rearrange("t n -> (t n)").unsqueeze(0)
        nc.sync.dma_start(out=vl_view[:, g * GIMG:(g + 1) * GIMG], in_=vls)

    # ------------------------------------------------------------------
    for t in range(T):
        while state["chunk"] * 2048 < min((t + 2) * NL * S, FTOT):
            emit_obs_chunk(state["chunk"])
            state["chunk"] += 1

        ob = (t * NL * S) % OBST_F
        pp = (t % 2) * PBUF  # ping-pong half of padded buffers

        for b in range(4):
            f0 = b * 432
            n = 432
            x1 = xact.tile([128, n], BF16, tag="x1")
            x2 = xact.tile([128, n], BF16, tag="x2")
            ps1 = ps_big.tile([128, n], F32, tag="ps")
            nc.tensor.matmul(ps1, c1w, obsT[:, ob + f0: ob + f0 + n],
                             start=True, stop=True)
            nc.scalar.activation(out=x1, in_=ps1, func=AF.Relu,
                                 bias=b_c1)
            ps2 = ps_big.tile([128, n], F32, tag="ps")
            nc.tensor.matmul(ps2, c2w, x1,
                             start=True, stop=True)
            nc.vector.tensor_scalar(out=x2, in0=ps2,
                                    scalar1=b_c2, scalar2=0.0,
                                    op0=ALU.add, op1=ALU.max)
            ps3 = ps_big.tile([8, n], F32, tag="ps")
            nc.tensor.matmul(ps3, c3w, x2,
                             start=True, stop=True)
            src = ps3.rearrange("c (i r w) -> c i r w", r=6, w=9)
            for dr in range(3):
                f1 = pp + PAD0 + 8 * b * PS - 10 * (dr - 1)
                dst = valid_ap(x4b, 32 * dr, 32 * dr + 8, f1, 8)
                if dr == 0:
                    nc.vector.tensor_scalar(out=dst, in0=src, scalar1=b_c3,
                                            scalar2=0.0, op0=ALU.add,
                                            op1=ALU.max)
                else:
                    nc.scalar.activation(out=dst, in_=src, func=AF.Relu,
                                         bias=b_c3)

        def conv3x3(xb, wmat, cs_out, bias_ap, out_cb, compact_dst=None,
                    evict_dve=False):
            for (p0, n, i0, nimg) in CONV_BLKS:
                pso = ps_big.tile([cs_out, 512], F32, tag="ps")
                for dc in range(3):
                    nc.tensor.matmul(
                        pso[:, 0:n], wmat[dc],
                        xb[:, pp + p0 + dc - 1: pp + p0 + dc - 1 + n],
                        start=(dc == 0), stop=(dc == 2))
                srcv = psum_valid(pso, cs_out, nimg)
                if compact_dst is not None:
                    gb2 = ((t // 8) % 2) * GIMG * S
                    ig = (t % 8) * NL + i0
                    v = compact_dst[:, gb2:gb2 + GIMG * S] \
                        .rearrange("c (rw i) -> c rw i", i=GIMG) \
                        .rearrange("c (r w) i -> c i r w", w=9)
                    dst = v[:, ig:ig + nimg, :, :]
                    nc.vector.tensor_scalar(out=dst, in0=srcv, scalar1=bias_ap,
                                            scalar2=0.0, op0=ALU.add,
                                            op1=ALU.max)
                else:
                    for dr in range(3):
                        f1 = pp + PAD0 + i0 * PS - 10 * (dr - 1)
                        dst = valid_ap(out_cb, 32 * dr, 32 * dr + cs_out,
                                       f1, nimg)
                        if evict_dve and dr == 0:
                            nc.vector.tensor_scalar(
                                out=dst, in0=srcv, scalar1=bias_ap,
                                scalar2=0.0, op0=ALU.add, op1=ALU.max)
                        else:
                            nc.scalar.activation(out=dst, in_=srcv,
                                                 func=AF.Relu, bias=bias_ap)

        conv3x3(x4b, [w4[:, dc, :] for dc in range(3)], 16, b_c4, x5b,
                evict_dve=True)
        if t >= GRP:
            gru_step(t - GRP)
        conv3x3(x5b, [w5[:, dc, :] for dc in range(3)], 32, b_c5, x6b,
                evict_dve=True)
        conv3x3(x6b, [w6[:, dc, :] for dc in range(3)], 32, b_c6, None,
                compact_dst=dbuf)

        if t % GRP == GRP - 1:
            g = t // GRP
            dense_group(g)
            if g >= 2:
                heads_group(g - 2)

    for tt in range(T - GRP, T):
        gru_step(tt)
    heads_group(6)
    heads_group(7)
    fh_view = d["fh_out"].rearrange("n h -> h n")
    nc.sync.dma_start(out=fh_view, in_=grub[:, (T - 1) * NL: T * NL])

    for p in reversed(ctxs):
        p.__exit__(None, None, None)


# ---------------------------------------------------------------------------
_NC_CACHE = {}


def _get_nc():
    if "nc" not in _NC_CACHE:
        _NC_CACHE["nc"] = build_bass()
    return _NC_CACHE["nc"]


def _prep_weights(kw):
    f = np.float32
    kw = {k: np.asarray(v) for k, v in kw.items()}
    out = {}
    out["c1w"] = kw["c1w"].reshape(26, 128).astype(BF)
    out["c2w"] = kw["c2w"].reshape(128, 128).astype(BF)
    out["c3w"] = kw["c3w"].reshape(128, 8).astype(BF)
    for nm, key, ci, co in (("w4", "c4w", 8, 16), ("w5", "c5w", 16, 32),
                            ("w6", "c6w", 32, 32)):
        w = np.zeros((96, 3, co), f)
        src = kw[key]
        for dc in range(3):
            for dr in range(3):
                w[32 * dr: 32 * dr + ci, dc, :] = src[dr, dc]
        out[nm] = w.astype(BF)
    out["dwT"] = kw["dw"].reshape(6, 9, 32, 128).transpose(2, 0, 1, 3) \
                          .reshape(32, S, 128).astype(BF).copy()
    lns, lnb = kw["lns"].astype(f), kw["lnb"].astype(f)
    out["wizp"] = (kw["wizk"] * lns[:, None]).astype(f)
    out["wirp"] = (kw["wirk"] * lns[:, None]).astype(f)
    out["wihp"] = (kw["wihk"] * lns[:, None]).astype(f)
    for k in ("whz", "whr", "whh", "afw", "aow", "cfw", "cow"):
        out[k] = kw[k].astype(f)
    bz = (kw["bz"] + kw["wizk"].T @ lnb).astype(f)
    br = (kw["br"] + kw["wirk"].T @ lnb).astype(f)
    bh = (kw["bh"] + kw["wihk"].T @ lnb).astype(f)
    bias = np.zeros((128, 16), f)
    for k, j in (("c1b", 0), ("c2b", 1), ("c3b", 2), ("c4b", 3), ("c5b", 4),
                 ("c6b", 5), ("db", 6)):
        v = kw[k]
        bias[: v.shape[0], j] = v
    bias[:, 7], bias[:, 8], bias[:, 9] = bz, br, bh
    bias[:, 10], bias[:, 11] = kw["afb"], kw["cfb"]
    bias[:6, 12] = kw["aob"]
    bias[:1, 13] = kw["cob"]
    out["biases"] = bias
    out["ident"] = np.eye(128, dtype=f)
    out["identb"] = np.eye(128, dtype=BF)
    out["ones1x"] = np.ones((1, 128), f)
    out["onecol"] = np.ones((128, 1), f)
    return out


def make_in_maps(inputs):
    wmap = _prep_weights({k: v for k, v in inputs.items()
                          if k not in ("obs", "hidden", "dones")})
    obs = np.asarray(inputs["obs"], np.float32)
    hidden = np.asarray(inputs["hidden"], np.float32)
    dones = np.asarray(inputs["dones"])
    in_maps = []
    for c in range(8):
        sl = slice(c * NL, (c + 1) * NL)
        m = dict(wmap)
        m["obs"] = np.ascontiguousarray(obs[:, sl]).reshape(FTOT, CIN)
        m["mask1"] = np.ascontiguousarray(
            1.0 - dones[:, sl].astype(np.float32)).reshape(1, IMG)
        m["h0"] = np.ascontiguousarray(hidden[sl].T)
        in_maps.append(m)
    return in_maps


def kernel(**inputs):
    from concourse.bass_utils import run_bass_kernel_spmd

    nc = _get_nc()
    in_maps = make_in_maps(inputs)
    res = run_bass_kernel_spmd(nc, in_maps, core_ids=list(range(8)))
    outs = res.results
    fh = np.concatenate([outs[c]["final_hidden"] for c in range(8)], 0)
    lg = np.concatenate([outs[c]["logits"] for c in range(8)], 1)
    vl = np.concatenate([outs[c]["value"] for c in range(8)], 1)
    return fh, lg, vl


if __name__ == "__main__":
    nc = build_bass()
    print("build ok")


# revision 24
# speedup vs baseline: 1.4035x; 1.0001x over previous
"""Trainium2 Bass kernel for ActorCriticRNN (8-core data-parallel over actors).

Per-core shard: 32 actors, T=64 steps, 2048 images of 6x9x26.
Pipeline (channel-major activations [C, positions]):
  obs --PE-transpose--> [26, pos] -> 1x1 convs c1,c2,c3 (matmul over C) ->
  3x3 convs c4,c5,c6 as: dr-taps stacked into 32-aligned partition strips at
  relu-eviction time, dc-taps as 3 PSUM-accumulated matmuls with shifted rhs
  over a zero-padded free layout (7 rows x 10 cols per image, shared pads) ->
  dense (54 accumulated K=32 matmuls) + LayerNorm (over partitions, via
  ones-matmul stats + K=1 broadcast matmuls) -> GRU (1 step emitted per conv
  timestep, 8 steps behind) -> actor/critic heads per 8-step group.
float32 storage; float32r matmuls (full PE rate at N>=256).
"""

import sys

for _p in ("/opt/trn_rl_repo",):
    if _p not in sys.path:
        sys.path.insert(0, _p)

import numpy as np
import ml_dtypes
BF = ml_dtypes.bfloat16

import concourse.bass as bass
import concourse.bacc as bacc
import concourse.mybir as mybir
import concourse.tile as tile

F32 = mybir.dt.float32
F32R = mybir.dt.float32r
BF16 = mybir.dt.bfloat16
AF = mybir.ActivationFunctionType
ALU = mybir.AluOpType

T, NL, HH, WW, CIN = 64, 32, 6, 9, 26
IMG = T * NL            # 2048
S = HH * WW             # 54
FTOT = IMG * S          # 110592
PW, PH = 10, 7
PS = PW * PH            # 70
PAD0 = 12
TPS = NL * PS           # 2240
PBUF = PAD0 + TPS + 12  # 2264
HID = 128
GRP = 8
GIMG = GRP * NL         # 256
OBST_F = 4 * NL * S     # 6912 (4-timestep ring)
DB_F = 2 * GIMG * S     # 27648 (2-group ring)
CONV_BLKS = [(12 + 490 * k, min(490, 2240 - 490 * k), 7 * k,
              min(7, NL - 7 * k)) for k in range(5)]


def _r(ap):
    return ap.bitcast(F32R)


def build_bass():
    nc = bacc.Bacc(None, target_bir_lowering=False)

    def din(name, shape, dt=F32):
        return nc.declare_dram_parameter(name, list(shape), dt, isOutput=False)

    d = {}
    d["obs"] = din("obs", [FTOT, CIN])
    d["mask1"] = din("mask1", [1, IMG])
    d["h0"] = din("h0", [HID, NL])
    d["c1w"] = din("c1w", [CIN, 128], BF16)
    d["c2w"] = din("c2w", [128, 128], BF16)
    d["c3w"] = din("c3w", [128, 8], BF16)
    d["w4"] = din("w4", [96, 3, 16], BF16)
    d["w5"] = din("w5", [96, 3, 32], BF16)
    d["w6"] = din("w6", [96, 3, 32], BF16)
    d["dwT"] = din("dwT", [32, S, HID], BF16)
    d["biases"] = din("biases", [128, 16])
    for k in ("wizp", "wirp", "wihp", "whz", "whr", "whh", "afw", "cfw"):
        d[k] = din(k, [HID, HID])
    d["aow"] = din("aow", [HID, 6])
    d["cow"] = din("cow", [HID, 1])
    d["ident"] = din("ident", [128, 128])
    d["identb"] = din("identb", [128, 128], BF16)
    d["ones1x"] = din("ones1x", [1, 128])
    d["onecol"] = din("onecol", [128, 1])
    d["fh_out"] = nc.declare_dram_parameter("final_hidden", [NL, HID], F32,
                                            isOutput=True)
    d["lg_out"] = nc.declare_dram_parameter("logits", [T, NL, 6], F32,
                                            isOutput=True)
    d["vl_out"] = nc.declare_dram_parameter("value", [T, NL], F32,
                                            isOutput=True)

    with tile.TileContext(nc) as tc, \
            nc.allow_low_precision(reason="bf16 mid-layers within rel-err budget"):
        build_body(nc, tc, d)
    if not nc.is_finalized():
        nc.finalize()
    return nc


def build_body(nc, tc, d):
    ctxs = []

    def pool(name, bufs, space="SBUF"):
        p = tc.tile_pool(name=name, bufs=bufs, space=space)
        ctxs.append(p)
        return p.__enter__()

    wpool = pool("weights", 1)
    persist = pool("persist", 1)
    obsp = pool("obs_in", 4)
    xact = pool("xact", 4)
    embp = pool("emb", 3)
    smallp = pool("small", 4)
    outp = pool("outs", 1)
    ps_big = pool("ps_big", 6, space="PSUM")
    ps_sm = pool("ps_sm", 2, space="PSUM")

    def wload(name, shape, dt=F32, rr=False):
        t = wpool.tile(list(shape), dt, tag=name)
        nc.sync.dma_start(out=_r(t) if rr else t,
                          in_=_r(d[name][:]) if rr else d[name][:])
        return t

    c1w = wload("c1w", [CIN, 128], BF16)
    c2w = wload("c2w", [128, 128], BF16)
    c3w = wload("c3w", [128, 8], BF16)
    w4 = wload("w4", [96, 3, 16], BF16)
    w5 = wload("w5", [96, 3, 32], BF16)
    w6 = wload("w6", [96, 3, 32], BF16)
    dwT = wload("dwT", [32, S, HID], BF16)
    bia = wload("biases", [128, 16])
    wizp = wload("wizp", [HID, HID], rr=True)
    wirp = wload("wirp", [HID, HID], rr=True)
    wihp = wload("wihp", [HID, HID], rr=True)
    whz = wload("whz", [HID, HID], rr=True)
    whr = wload("whr", [HID, HID], rr=True)
    whh = wload("whh", [HID, HID], rr=True)
    afw = wload("afw", [HID, HID], rr=True)
    aow = wload("aow", [HID, 6], rr=True)
    cfw = wload("cfw", [HID, HID], rr=True)
    cow = wload("cow", [HID, 1], rr=True)
    ident = wload("ident", [128, 128])
    identb = wload("identb", [128, 128], BF16)
    ones1x = wload("ones1x", [1, 128], rr=True)
    onecol = wload("onecol", [128, 1], rr=True)
    epst = wpool.tile([1, 1], F32, tag="epst")
    nc.vector.memset(epst, 1e-6)

    b_c1, b_c2, b_c3 = bia[:, 0:1], bia[:, 1:2], bia[:8, 2:3]
    b_c4, b_c5, b_c6 = bia[:16, 3:4], bia[:32, 4:5], bia[:32, 5:6]
    b_d = bia[:, 6:7]
    b_z, b_r, b_h = bia[:, 7:8], bia[:, 8:9], bia[:, 9:10]
    b_af, b_cf = bia[:, 10:11], bia[:, 11:12]
    b_ao, b_co = bia[:6, 12:13], bia[:1, 13:14]

    obsT = persist.tile([CIN, OBST_F], BF16)
    dbuf = persist.tile([32, DB_F], BF16)
    x4b = persist.tile([96, 2 * PBUF], BF16)
    x5b = persist.tile([96, 2 * PBUF], BF16)
    x6b = persist.tile([96, 2 * PBUF], BF16)
    wizb = persist.tile([HID, IMG], BF16)
    wirb = persist.tile([HID, IMG], BF16)
    wihb = persist.tile([HID, IMG], BF16)
    grub = persist.tile([HID, IMG], F32)
    maskb = persist.tile([HID, IMG], F32)
    h0sb = persist.tile([HID, NL], F32)
    msk1 = persist.tile([1, IMG], F32)

    nc.sync.dma_start(out=h0sb, in_=d["h0"][:])
    nc.sync.dma_start(out=_r(msk1), in_=_r(d["mask1"][:]))

    # one-time pad zeroing of both halves of the padded ping-pong buffers:
    # valid-cell evictions never touch pad cells, so zeros persist.
    for buf in (x4b, x5b, x6b):
        nc.vector.memset(buf, 0.0)

    # mask broadcast to all 128 partitions via K=1 matmuls
    for q in range(IMG // 512):
        psm = ps_big.tile([128, 512], F32, tag="ps")
        nc.tensor.matmul(psm, _r(ones1x), _r(msk1[:, q * 512:(q + 1) * 512]),
                         start=True, stop=True)
        nc.scalar.activation(out=maskb[:, q * 512:(q + 1) * 512], in_=psm,
                             func=AF.Copy)

    obs_r = d["obs"].rearrange("(a j p) c -> a p j c", j=16, p=128)
    state = {"chunk": 0}

    def emit_obs_chunk(a):
        ot = obsp.tile([128, 16, CIN], F32, tag="ot")
        nc.sync.dma_start(out=ot, in_=obs_r[a])
        otb = obsp.tile([128, 16, CIN], BF16, tag="otb")
        nc.vector.tensor_copy(otb, ot)
        for q in range(4):
            pst = ps_sm.tile([26, 512], BF16, tag="pss")
            for j in range(4):
                nc.tensor.transpose(pst[:, j * 128:(j + 1) * 128],
                                    otb[:, q * 4 + j, :], identb)
            base = (a * 2048 + q * 512) % OBST_F
            if base + 512 <= OBST_F:
                nc.vector.tensor_copy(obsT[:, base: base + 512], pst)
            else:
                r = OBST_F - base
                nc.vector.tensor_copy(obsT[:, base: base + r], pst[:, 0:r])
                nc.vector.tensor_copy(obsT[:, 0: 512 - r], pst[:, r:512])

    def valid_ap(buf, plo, phi, f0, nimg):
        v = buf[plo:phi, f0:f0 + nimg * PS]
        return v.rearrange("c (i r w) -> c i r w", r=PH, w=PW)[:, :, 0:6, 0:9]

    def psum_valid(ps, cs, nimg):
        return ps[0:cs, 0:nimg * PS] \
            .rearrange("c (i r w) -> c i r w", r=PH, w=PW)[:, :, 0:6, 0:9]

    def gru_step(tt):
        c0 = tt * NL
        hprev = h0sb if tt == 0 else grub[:, (tt - 1) * NL: tt * NL]
        hm = smallp.tile([HID, NL], F32, tag="hm")
        nc.vector.tensor_mul(_r(hm), hprev, maskb[:, c0:c0 + NL])
        psz = ps_sm.tile([HID, NL], F32, tag="pss")
        nc.tensor.matmul(psz, _r(whz), _r(hm), start=True, stop=False)
        nc.tensor.matmul(psz, identb, wizb[:, c0:c0 + NL],
                         start=False, stop=True)
        psr = ps_sm.tile([HID, NL], F32, tag="pss")
        nc.tensor.matmul(psr, _r(whr), _r(hm), start=True, stop=False)
        nc.tensor.matmul(psr, identb, wirb[:, c0:c0 + NL],
                         start=False, stop=True)
        zz = smallp.tile([HID, NL], F32, tag="zz")
        nc.scalar.activation(out=zz, in_=psz, func=AF.Sigmoid, bias=b_z)
        rr = smallp.tile([HID, NL], F32, tag="rr")
        nc.scalar.activation(out=rr, in_=psr, func=AF.Sigmoid, bias=b_r)
        rh = smallp.tile([HID, NL], F32, tag="rh")
        nc.vector.tensor_mul(_r(rh), rr, hm)
        psh = ps_sm.tile([HID, NL], F32, tag="pss")
        nc.tensor.matmul(psh, _r(whh), _r(rh), start=True, stop=False)
        nc.tensor.matmul(psh, identb, wihb[:, c0:c0 + NL],
                         start=False, stop=True)
        hh = smallp.tile([HID, NL], F32, tag="hh")
        nc.scalar.activation(out=hh, in_=psh, func=AF.Tanh, bias=b_h)
        d1 = smallp.tile([HID, NL], F32, tag="d1")
        nc.vector.tensor_sub(d1, hh, hm)
        nc.vector.tensor_mul(d1, zz, d1)
        nc.vector.tensor_add(_r(grub[:, c0:c0 + NL]), hm, d1)

    def dense_group(g):
        gb = (g % 2) * GIMG * S
        emb = embp.tile([HID, GIMG], F32, tag="emb")
        psd = ps_big.tile([HID, GIMG], F32, tag="ps")
        for rc in range(S):
            nc.tensor.matmul(psd, dwT[:, rc, :],
                             dbuf[:, gb + rc * GIMG: gb + (rc + 1) * GIMG],
                             start=(rc == 0), stop=(rc == S - 1))
        nc.scalar.activation(out=_r(emb), in_=psd, func=AF.Relu, bias=b_d)

        sq = embp.tile([HID, GIMG], F32, tag="sq")
        nc.vector.tensor_mul(_r(sq), emb, emb)
        ps_s1 = ps_sm.tile([1, GIMG], F32, tag="pss")
        ps_s2 = ps_sm.tile([1, GIMG], F32, tag="pss")
        nc.tensor.matmul(ps_s1, _r(onecol), _r(emb), start=True, stop=True)
        nc.tensor.matmul(ps_s2, _r(onecol), _r(sq), start=True, stop=True)
        mu = smallp.tile([1, GIMG], F32, tag="mu")
        va = smallp.tile([1, GIMG], F32, tag="va")
        aa = smallp.tile([1, GIMG], F32, tag="aa")
        bb = smallp.tile([1, GIMG], F32, tag="bb")
        nc.vector.tensor_scalar_mul(mu, ps_s1, 1.0 / HID)
        nc.vector.tensor_scalar_mul(va, ps_s2, 1.0 / HID)
        nc.vector.tensor_mul(_r(bb), mu, mu)
        nc.vector.tensor_sub(va, va, bb)
        nc.scalar.activation(out=va, in_=va, func=AF.Sqrt, bias=epst)
        nc.vector.reciprocal(_r(aa), va)
        nc.vector.tensor_mul(_r(bb), mu, aa)
        nc.vector.tensor_scalar_mul(_r(bb), bb, -1.0)
        ps_a = ps_sm.tile([HID, GIMG], F32, tag="pss")
        ps_b = ps_sm.tile([HID, GIMG], F32, tag="pss")
        nc.tensor.matmul(ps_a, _r(ones1x), _r(aa), start=True, stop=True)
        nc.tensor.matmul(ps_b, _r(ones1x), _r(bb), start=True, stop=True)
        nc.vector.tensor_mul(_r(emb), emb, ps_a)
        nc.vector.tensor_add(_r(emb), emb, ps_b)

        for wmat, dstb in ((wizp, wizb), (wirp, wirb), (wihp, wihb)):
            psp = ps_big.tile([HID, GIMG], F32, tag="ps")
            nc.tensor.matmul(psp, _r(wmat), _r(emb), start=True, stop=True)
            nc.scalar.activation(out=dstb[:, g * GIMG:(g + 1) * GIMG],
                                 in_=psp, func=AF.Copy)

    def heads_group(g):
        gsl = grub[:, g * GIMG:(g + 1) * GIMG]
        psa = ps_big.tile([HID, GIMG], F32, tag="ps")
        nc.tensor.matmul(psa, _r(afw), _r(gsl), start=True, stop=True)
        act1 = embp.tile([HID, GIMG], F32, tag="act1")
        nc.scalar.activation(out=_r(act1), in_=psa, func=AF.Relu, bias=b_af)
        psl = ps_sm.tile([6, GIMG], F32, tag="pss")
        nc.tensor.matmul(psl, _r(aow), _r(act1), start=True, stop=True)
        lgs = outp.tile([6, GIMG], F32, tag="lgs")
        nc.vector.tensor_scalar_add(lgs, psl, b_ao)
        lg_view = d["lg_out"].rearrange("t n a -> a (t n)")
        nc.sync.dma_start(out=lg_view[:, g * GIMG:(g + 1) * GIMG], in_=lgs)

        psc = ps_big.tile([HID, GIMG], F32, tag="ps")
        nc.tensor.matmul(psc, _r(cfw), _r(gsl), start=True, stop=True)
        crt1 = embp.tile([HID, GIMG], F32, tag="act1")
        nc.scalar.activation(out=_r(crt1), in_=psc, func=AF.Relu, bias=b_cf)
        psv = ps_sm.tile([1, GIMG], F32, tag="pss")
        nc.tensor.matmul(psv, _r(cow), _r(crt1), start=True, stop=True)
        vls = outp.tile([1, GIMG], F32, tag="vls")
        nc.vector.tensor_scalar_add(vls, psv, b_co)
        vl_view = d["vl_out"].rearrange("t n -> (t n)").unsqueeze(0)
        nc.sync.dma_start(out=vl_view[:, g * GIMG:(g + 1) * GIMG], in_=vls)

    # ------------------------------------------------------------------
    for t in range(T):
        while state["chunk"] * 2048 < min((t + 2) * NL * S, FTOT):
            emit_obs_chunk(state["chunk"])
            state["chunk"] += 1

        ob = (t * NL * S) % OBST_F
        pp = (t % 2) * PBUF  # ping-pong half of padded buffers

        for b in range(4):
            f0 = b * 432
            n = 432
            x1 = xact.tile([128, n], BF16, tag="x1")
            x2 = xact.tile([128, n], BF16, tag="x2")
            ps1 = ps_big.tile([128, n], F32, tag="ps")
            nc.tensor.matmul(ps1, c1w, obsT[:, ob + f0: ob + f0 + n],
                             start=True, stop=True)
            nc.scalar.activation(out=x1, in_=ps1, func=AF.Relu,
                                 bias=b_c1)
            ps2 = ps_big.tile([128, n], F32, tag="ps")
            nc.tensor.matmul(ps2, c2w, x1,
                             start=True, stop=True)
            nc.vector.tensor_scalar(out=x2, in0=ps2,
                                    scalar1=b_c2, scalar2=0.0,
                                    op0=ALU.add, op1=ALU.max)
            ps3 = ps_big.tile([8, n], F32, tag="ps")
            nc.tensor.matmul(ps3, c3w, x2,
                             start=True, stop=True)
            src = ps3.rearrange("c (i r w) -> c i r w", r=6, w=9)
            for dr in range(3):
                f1 = pp + PAD0 + 8 * b * PS - 10 * (dr - 1)
                dst = valid_ap(x4b, 32 * dr, 32 * dr + 8, f1, 8)
                if dr == 0:
                    nc.vector.tensor_scalar(out=dst, in0=src, scalar1=b_c3,
                                            scalar2=0.0, op0=ALU.add,
                                            op1=ALU.max)
                else:
                    nc.scalar.activation(out=dst, in_=src, func=AF.Relu,
                                         bias=b_c3)

        def conv3x3(xb, wmat, cs_out, bias_ap, out_cb, compact_dst=None,
                    evict_dve=False):
            for (p0, n, i0, nimg) in CONV_BLKS:
                pso = ps_big.tile([cs_out, 512], F32, tag="ps")
                for dc in range(3):
                    nc.tensor.matmul(
                        pso[:, 0:n], wmat[dc],
                        xb[:, pp + p0 + dc - 1: pp + p0 + dc - 1 + n],
                        start=(dc == 0), stop=(dc == 2))
                srcv = psum_valid(pso, cs_out, nimg)
                if compact_dst is not None:
                    gb2 = ((t // 8) % 2) * GIMG * S
                    ig = (t % 8) * NL + i0
                    v = compact_dst[:, gb2:gb2 + GIMG * S] \
                        .rearrange("c (rw i) -> c rw i", i=GIMG) \
                        .rearrange("c (r w) i -> c i r w", w=9)
                    dst = v[:, ig:ig + nimg, :, :]
                    nc.vector.tensor_scalar(out=dst, in0=srcv, scalar1=bias_ap,
                                            scalar2=0.0, op0=ALU.add,
                                            op1=ALU.max)
                else:
                    for dr in range(3):
                        f1 = pp + PAD0 + i0 * PS - 10 * (dr - 1)
                        dst = valid_ap(out_cb, 32 * dr, 32 * dr + cs_out,
                                       f1, nimg)
                        if evict_dve and dr == 0:
                            nc.vector.tensor_scalar(
                                out=dst, in0=srcv, scalar1=bias_ap,
                                scalar2=0.0, op0=ALU.add, op1=ALU.max)
                        else:
                            nc.scalar.activation(out=dst, in_=srcv,
                                                 func=AF.Relu, bias=bias_ap)

        conv3x3(x4b, [w4[:, dc, :] for dc in range(3)], 16, b_c4, x5b,
                evict_dve=True)
        if t >= GRP:
            gru_step(t - GRP)
        conv3x3(x5b, [w5[:, dc, :] for dc in range(3)], 32, b_c5, x6b,
                evict_dve=True)
        conv3x3(x6b, [w6[:, dc, :] for dc in range(3)], 32, b_c6, None,
                compact_dst=dbuf)

        if t % GRP == GRP - 1:
            g = t // GRP
            dense_group(g)
            if g >= 2:
                heads_group(g - 2)

    for tt in range(T - GRP, T):
        gru_step(tt)
    heads_group(6)
    heads_group(7)
    fh_view = d["fh_out"].rearrange("n h -> h n")
    nc.sync.dma_start(out=fh_view, in_=grub[:, (T - 1) * NL: T * NL])

    for p in reversed(ctxs):
        p.__exit__(None, None, None)


# ---------------------------------------------------------------------------
_NC_CACHE = {}


def _get_nc():
    if "nc" not in _NC_CACHE:
        _NC_CACHE["nc"] = build_bass()
    return _NC_CACHE["nc"]


def _prep_weights(kw):
    f = np.float32
    kw = {k: np.asarray(v) for k, v in kw.items()}
    out = {}
    out["c1w"] = kw["c1w"].reshape(26, 128).astype(BF)
    out["c2w"] = kw["c2w"].reshape(128, 128).astype(BF)
    out["c3w"] = kw["c3w"].reshape(128, 8).astype(BF)
    for nm, key, ci, co in (("w4", "c4w", 8, 16), ("w5", "c5w", 16, 32),
                            ("w6", "c6w", 32, 32)):
        w = np.zeros((96, 3, co), f)
        src = kw[key]
        for dc in range(3):
            for dr in range(3):
                w[32 * dr: 32 * dr + ci, dc, :] = src[dr, dc]
        out[nm] = w.astype(BF)
    out["dwT"] = kw["dw"].reshape(6, 9, 32, 128).transpose(2, 0, 1, 3) \
                          .reshape(32, S, 128).astype(BF).copy()
    lns, lnb = kw["lns"].astype(f), kw["lnb"].astype(f)
    out["wizp"] = (kw["wizk"] * lns[:, None]).astype(f)
    out["wirp"] = (kw["wirk"] * lns[:, None]).astype(f)
    out["wihp"] = (kw["wihk"] * lns[:, None]).astype(f)
    for k in ("whz", "whr", "whh", "afw", "aow", "cfw", "cow"):
        out[k] = kw[k].astype(f)
    bz = (kw["bz"] + kw["wizk"].T @ lnb).astype(f)
    br = (kw["br"] + kw["wirk"].T @ lnb).astype(f)
    bh = (kw["bh"] + kw["wihk"].T @ lnb).astype(f)
    bias = np.zeros((128, 16), f)
    for k, j in (("c1b", 0), ("c2b", 1), ("c3b", 2), ("c4b", 3), ("c5b", 4),
                 ("c6b", 5), ("db", 6)):
        v = kw[k]
        bias[: v.shape[0], j] = v
    bias[:, 7], bias[:, 8], bias[:, 9] = bz, br, bh
    bias[:, 10], bias[:, 11] = kw["afb"], kw["cfb"]
    bias[:6, 12] = kw["aob"]
    bias[:1, 13] = kw["cob"]
    out["biases"] = bias
    out["ident"] = np.eye(128, dtype=f)
    out["identb"] = np.eye(128, dtype=BF)
    out["ones1x"] = np.ones((1, 128), f)
    out["onecol"] = np.ones((128, 1), f)
    return out


def make_in_maps(inputs):
    wmap = _prep_weights({k: v for k, v in inputs.items()
                          if k not in ("obs", "hidden", "dones")})
    obs = np.asarray(inputs["obs"], np.float32)
    hidden = np.asarray(inputs["hidden"], np.float32)
    dones = np.asarray(inputs["dones"])
    in_maps = []
    for c in range(8):
        sl = slice(c * NL, (c + 1) * NL)
        m = dict(wmap)
        m["obs"] = np.ascontiguousarray(obs[:, sl]).reshape(FTOT, CIN)
        m["mask1"] = np.ascontiguousarray(
            1.0 - dones[:, sl].astype(np.float32)).reshape(1, IMG)
        m["h0"] = np.ascontiguousarray(hidden[sl].T)
        in_maps.append(m)
    return in_maps


def kernel(**inputs):
    from concourse.bass_utils import run_bass_kernel_spmd

    nc = _get_nc()
    in_maps = make_in_maps(inputs)
    res = run_bass_kernel_spmd(nc, in_maps, core_ids=list(range(8)))
    outs = res.results
    fh = np.concatenate([outs[c]["final_hidden"] for c in range(8)], 0)
    lg = np.concatenate([outs[c]["logits"] for c in range(8)], 1)
    vl = np.concatenate([outs[c]["value"] for c in range(8)], 1)
    return fh, lg, vl


if __name__ == "__main__":
    nc = build_bass()
    print("build ok")
